# revision 14
# baseline (speedup 1.0000x reference)
"""Trainium2 Bass kernel for a CrossAttentionBlock (GroupNorm + 8-head
cross-attention + output projection + residual).

Sharding: one attention head per NeuronCore (8 heads / 8 cores).  Each core
computes its head's partial output projection wo[:, h] @ attn_h; the host sums
the 8 partials (partial-sum unshard).  Residual and output bias are added on
core 0 only (mask input), so the host-side sum is a pure reduce.

v3: fp8e4 DoubleRow matmuls + multi-engine softmax exp.
 - x arrives bf16 (Q proj in bf16, GroupNorm stats, residual); context and
   attention weights arrive fp8e4, pre-scaled by 8 on the host so w*8 clears
   the e4m3 subnormal threshold.  Scale corrections fold into the exp scale
   and the output epilogue constant.
 - Scores tt = 2*k8^T q8 via stride-0 broadcast of the DoubleRow k-subtile
   dim; vT is computed directly as ctx8^T wv8 (keys on partitions) with a
   ones column riding the AV matmul as the softmax denominator (stationary
   width padded to 96 - DoubleRow needs a multiple of 32).
 - K/vT PSUM evacuation runs on ScalarE (activation Identity/Copy) so the
   whole K/V production overlaps the DVE GroupNorm-stats window; the
   residual+bias epilogue tiles (xm) are precomputed on GpSimd up front.
 - exp runs on ScalarE for 11/16 groups per chunk; the other 5 use a
   bit-trick path (DVE: b = tt*A + B in f32; GpSimd: f32->uint8 round, bits
   reinterpreted as e4m3 ~ exp) to spread the elementwise softmax cost over
   three engines.  The PWL error is a global constant times +-2.6% ripple;
   the constant cancels in softmax.

Self-contained: hardcodes all shapes from the problem spec.
"""

import math
import sys

sys.path.insert(0, "/opt/trn_rl_repo")

import numpy as np

import concourse.bass as bass
import concourse.tile as tile
from concourse import bacc, mybir

F32 = mybir.dt.float32
F32R = mybir.dt.float32r
BF16 = mybir.dt.bfloat16
FP8 = mybir.dt.float8e4
U8 = mybir.dt.uint8
DR = mybir.MatmulPerfMode.DoubleRow
AF = mybir.ActivationFunctionType

CH = 512          # x channels
CTXC = 768        # context channels
N = 4096          # spatial positions (64*64)
NH = 8            # heads
DH = 64           # head dim
G = 32            # groupnorm groups
EPS = 1e-5
NCO = CH // 128   # x channel blocks (4)
NCK = CTXC // 128  # ctx channel blocks (6)
IC = 512          # query-chunk size
NIC = N // IC     # 8 query chunks
NJT = N // 128    # 32 key tiles
NG = NJT // 2     # 16 exp groups (2 key tiles each) per chunk
SCALE = 1.0 / 8.0  # 1/sqrt(DH)
W8 = 8.0           # host-side fp8 weight pre-scale
# tt = 2 * (8 wk)^T q = 16 * (k.q); true score = SCALE * (k.q)
EXP_SCALE = SCALE / 16.0
# attn8 = C0 * pav[0:64] / den = C0 * 8 * sum(p v); attn8 ~ 1024 * sum(p v)
C0 = 128.0
# po = 2 * (8 wo)^T attn8 = 2*8*8*128 * o_true
O_SCALE = 1.0 / 16384.0
# bit-trick exp: uint8 bits of e4m3 ~ 8*(log2(v)+7); round mode on device
TRICK_A = EXP_SCALE * 8.0 / math.log(2.0)
TRICK_B = 55.8
# groups per chunk routed to the DVE+Pool bit-trick path
TRICK_GROUPS = frozenset({1, 4, 7, 10, 13})

ADD = mybir.AluOpType.add
SUB = mybir.AluOpType.subtract
MUL = mybir.AluOpType.mult


def bc2(ap, p, n):
    """Insert a stride-0 k-subtile dim of size 2 (DoubleRow dup trick)."""
    return ap.rearrange("p (o n) -> p o n", o=1).to_broadcast((p, 2, n))


def build_nc():
    nc = bacc.Bacc("TRN2", num_devices=8, debug=False)

    xb = nc.dram_tensor("xb", (CH, N), BF16, kind="ExternalInput")
    ctx8 = nc.dram_tensor("ctx8", (CTXC, N), FP8, kind="ExternalInput")
    gn_w = nc.dram_tensor("gn_w", (CH,), F32, kind="ExternalInput")
    gn_b = nc.dram_tensor("gn_b", (CH,), F32, kind="ExternalInput")
    wqT = nc.dram_tensor("wqT", (CH, DH), F32, kind="ExternalInput")
    wkT8 = nc.dram_tensor("wkT8", (CTXC, DH), FP8, kind="ExternalInput")
    wvT8 = nc.dram_tensor("wvT8", (CTXC, DH), FP8, kind="ExternalInput")
    woT8 = nc.dram_tensor("woT8", (DH, CH), FP8, kind="ExternalInput")
    woT = nc.dram_tensor("woT", (DH, CH), F32, kind="ExternalInput")
    bq = nc.dram_tensor("bq", (DH,), F32, kind="ExternalInput")
    bk8 = nc.dram_tensor("bk8", (DH,), F32, kind="ExternalInput")
    bv = nc.dram_tensor("bv", (DH,), F32, kind="ExternalInput")
    bo = nc.dram_tensor("bo", (CH,), F32, kind="ExternalInput")
    is0 = nc.dram_tensor("is0", (1, 1), F32, kind="ExternalInput")
    gmat = nc.dram_tensor("gmat", (128, 8), F32, kind="ExternalInput")
    gmatT = nc.dram_tensor("gmatT", (8, 128), F32, kind="ExternalInput")
    vcol = nc.dram_tensor("vcol", (128, 32), FP8, kind="ExternalInput")
    partial = nc.dram_tensor("partial", (CH, N), BF16, kind="ExternalOutput")

    pvw = partial.rearrange("(co p) n -> p co n", p=128)

    with tile.TileContext(nc) as tc:
        singles = tc.alloc_tile_pool(name="singles", bufs=1)
        # PSUM: pst 2x2 + psav 2 + bsh 2 = 8 banks
        pst = tc.alloc_tile_pool(name="pst", bufs=2, space="PSUM")
        psav = tc.alloc_tile_pool(name="psav", bufs=2, space="PSUM")
        bsh = tc.alloc_tile_pool(name="bsh", bufs=2, space="PSUM")
        expp = tc.alloc_tile_pool(name="expp", bufs=4)
        bfp = tc.alloc_tile_pool(name="bfp", bufs=2)
        attp = tc.alloc_tile_pool(name="attp", bufs=2)
        outp = tc.alloc_tile_pool(name="outp", bufs=2)
        stats_pool = tc.alloc_tile_pool(name="stats", bufs=2)

        # ---------------- persistent SBUF tiles ----------------
        xb_sb = singles.tile([128, NCO, N], BF16)
        ctx8_sb = singles.tile([128, NCK, N], FP8)
        xm_sb = singles.tile([128, NCO, N], F32)
        q8 = singles.tile([DH, N], FP8)
        k8 = singles.tile([DH, N], FP8)
        vT8 = singles.tile([128, NG, 2, 96], FP8)
        wqT_sb = singles.tile([128, NCO, DH], F32)
        wqs_bf = singles.tile([128, NCO, DH], BF16)
        wkT8_sb = singles.tile([128, NCK, DH], FP8)
        wvT8_sb = singles.tile([128, NCK, DH], FP8)
        woT8_sb = singles.tile([DH, CH], FP8)
        woT_sb = singles.tile([DH, CH], F32)
        ones_c0 = singles.tile([1, DH], F32R)
        ones_f = singles.tile([1, DH], F32)
        gmat_sb = singles.tile([128, 8], F32)
        gmatT_sb = singles.tile([8, 128], F32)
        gnw_pc = singles.tile([128, NCO], F32)
        gnb_pc = singles.tile([128, NCO], F32)
        mvall = singles.tile([128, NCO, 2], F32)
        mv3 = singles.tile([128, NCO, 3], F32)
        gsm = singles.tile([8, NCO, 3], F32)
        gmu84 = singles.tile([8, NCO], F32)
        gvar84 = singles.tile([8, NCO], F32)
        srt84 = singles.tile([8, NCO], F32)
        grs = singles.tile([8, NCO, 2], F32)
        rg_pc = singles.tile([128, NCO, 2], F32)
        tmp_pc = singles.tile([128, NCO], F32)
        eps8 = singles.tile([8, 1], F32)
        a_pc = singles.tile([128, NCO], F32)
        d_pc = singles.tile([128, NCO], F32)
        qbias = singles.tile([DH, 1], F32)
        bq_sb = singles.tile([DH, 1], F32)
        kb_sb = singles.tile([DH, 1], F32)
        bv64 = singles.tile([DH, 1], F32)
        wobv_pc = singles.tile([128, NCO], F32)
        bomv = singles.tile([128, NCO], F32)
        bo_pc = singles.tile([128, NCO], F32)
        bom = singles.tile([128, NCO], F32)
        msk = singles.tile([128, 1], F32)
        expwarm = singles.tile([1, 1], F32)

        # ---------------- input loads ----------------
        xv = xb.rearrange("(co p) n -> p co n", p=128)
        cv = ctx8.rearrange("(ck p) n -> p ck n", p=128)
        for co in range(NCO):
            nc.sync.dma_start(out=xb_sb[:, co, :], in_=xv[:, co, :])
        for qt in range(4):
            cs = slice(qt * (N // 4), (qt + 1) * (N // 4))
            for ck in range(NCK):
                nc.sync.dma_start(out=ctx8_sb[:, ck, cs], in_=cv[:, ck, cs])
        nc.sync.dma_start(out=wqT_sb[:], in_=wqT.rearrange("(co p) d -> p co d", p=128))
        nc.sync.dma_start(out=wkT8_sb[:], in_=wkT8.rearrange("(ck p) d -> p ck d", p=128))
        nc.sync.dma_start(out=wvT8_sb[:], in_=wvT8.rearrange("(ck p) d -> p ck d", p=128))
        nc.sync.dma_start(out=woT8_sb[:], in_=woT8[:])
        nc.sync.dma_start(out=woT_sb[:], in_=woT[:])
        nc.sync.dma_start(out=gnw_pc[:], in_=gn_w.rearrange("(co p) -> p co", p=128))
        nc.sync.dma_start(out=gnb_pc[:], in_=gn_b.rearrange("(co p) -> p co", p=128))
        nc.sync.dma_start(out=gmat_sb[:], in_=gmat[:])
        nc.sync.dma_start(out=gmatT_sb[:], in_=gmatT[:])
        nc.sync.dma_start(out=bq_sb[:], in_=bq[:][:, None])
        nc.sync.dma_start(out=kb_sb[:], in_=bk8[:][:, None])
        nc.sync.dma_start(out=bv64[:], in_=bv[:][:, None])
        nc.sync.dma_start(out=bo_pc[:], in_=bo.rearrange("(co p) -> p co", p=128))
        nc.sync.dma_start(out=msk[:], in_=is0[:].to_broadcast((128, 1)))
        # vT pad columns 64:96 = [1, 0, ..., 0] via broadcast DMA
        nc.sync.dma_start(
            out=vT8[:, :, :, 64:96].rearrange("p a b n -> p (a b) n"),
            in_=vcol.rearrange("p (a n) -> p a n", a=1).to_broadcast(
                (128, NG * 2, 32)))

        # small constants + activation table warm (Ln+Exp share a set)
        nc.vector.memset(ones_f[:], C0)
        nc.vector.tensor_copy(out=ones_c0[:], in_=ones_f[:])
        nc.vector.memset(eps8[:], EPS)
        nc.vector.memset(expwarm[:], 1.0)
        nc.scalar.activation(out=expwarm[:], in_=expwarm[:], func=AF.Ln)
        nc.scalar.activation(out=expwarm[:], in_=expwarm[:], func=AF.Exp)

        # output-bias terms: bomv = bo*msk + wo@bv
        nc.vector.tensor_scalar_mul(out=bom[:], in0=bo_pc[:], scalar1=msk[:])
        for ot in range(NCO):
            pwb = bsh.tile([128, 1], F32, tag="b", name="pwb")
            nc.tensor.matmul(pwb[:], woT_sb[:, ot * 128:(ot + 1) * 128], bv64[:],
                             start=True, stop=True)
            nc.vector.tensor_copy(out=wobv_pc[:, ot:ot + 1], in_=pwb[:])
        nc.vector.tensor_tensor(out=bomv[:], in0=bom[:], in1=wobv_pc[:], op=ADD)

        # ---------------- K and vT production (PE + ScalarE) ----------------
        def kv_quarter(qt):
            for lc in range(2):
                ic = qt * 2 + lc
                sl = slice(ic * IC, (ic + 1) * IC)
                pk = bsh.tile([DH, IC], F32, tag="b", name="pk")
                for j in range(NCK // 2):
                    nc.tensor.matmul(pk[:], wkT8_sb[:, 2 * j:2 * j + 2, :],
                                     ctx8_sb[:, 2 * j:2 * j + 2, sl],
                                     start=(j == 0), stop=(j == NCK // 2 - 1),
                                     perf_mode=DR)
                with nc.allow_low_precision(reason="fp8 attention operand"):
                    nc.scalar.activation(out=k8[:, sl], in_=pk[:],
                                         func=AF.Identity, bias=kb_sb[:])
            for h in range(2):  # two 4-key-tile batches per quarter
                kt0 = qt * 8 + h * 4
                pvt = bsh.tile([128, 4, DH], F32, tag="b", name="pvt")
                for i in range(4):
                    ks = slice((kt0 + i) * 128, (kt0 + i + 1) * 128)
                    for j in range(NCK // 2):
                        nc.tensor.matmul(pvt[:, i, :],
                                         ctx8_sb[:, 2 * j:2 * j + 2, ks],
                                         wvT8_sb[:, 2 * j:2 * j + 2, :],
                                         start=(j == 0),
                                         stop=(j == NCK // 2 - 1),
                                         perf_mode=DR)
                dst = vT8[:, kt0 // 2:kt0 // 2 + 2, :, 0:DH].rearrange(
                    "p a b d -> p (a b) d")
                with nc.allow_low_precision(reason="fp8 attention operand"):
                    nc.scalar.activation(out=dst, in_=pvt[:], func=AF.Copy)

        for qt in range(4):
            kv_quarter(qt)

        # residual epilogue tiles, precomputed on GpSimd during startup
        for co in range(NCO):
            nc.gpsimd.tensor_scalar(out=xm_sb[:, co, :], in0=xb_sb[:, co, :],
                                    scalar1=msk[:], scalar2=bomv[:, co:co + 1],
                                    op0=MUL, op1=ADD)

        # ---------------- groupnorm statistics (DVE) ----------------
        for co in range(NCO):
            st = stats_pool.tile([128, 8, 6], F32)
            xvw = xb_sb[:, co, :].rearrange("p (s c) -> p s c", c=512)
            for s in range(8):
                nc.vector.bn_stats(out=st[:, s, :], in_=xvw[:, s, :])
            nc.vector.bn_aggr(out=mvall[:, co, :], in_=st[:])
        # group stats via tiny PE matmuls, batched over channel blocks
        nc.vector.tensor_copy(out=mv3[:, :, 0:2], in_=mvall[:])
        nc.vector.tensor_tensor(out=mv3[:, :, 2:3], in0=mvall[:, :, 0:1],
                                in1=mvall[:, :, 0:1], op=MUL)
        psg = bsh.tile([8, NCO, 3], F32, tag="b", name="psg")
        nc.tensor.matmul(psg.rearrange("g co s -> g (co s)"), gmat_sb[:],
                         mv3.rearrange("p co s -> p (co s)"),
                         start=True, stop=True)
        nc.vector.tensor_copy(out=gsm[:], in_=psg[:])
        nc.vector.tensor_scalar_mul(out=gmu84[:], in0=gsm[:, :, 0], scalar1=1.0 / 16.0)
        nc.vector.tensor_tensor(out=gvar84[:], in0=gsm[:, :, 1], in1=gsm[:, :, 2],
                                op=ADD)
        nc.vector.tensor_scalar_mul(out=gvar84[:], in0=gvar84[:], scalar1=1.0 / 16.0)
        nc.vector.tensor_tensor(out=srt84[:], in0=gmu84[:], in1=gmu84[:], op=MUL)
        nc.vector.tensor_tensor(out=gvar84[:], in0=gvar84[:], in1=srt84[:], op=SUB)
        # rstd = exp(-0.5*ln(var+eps)): stays on the Ln+Exp activation set
        nc.scalar.activation(out=srt84[:], in_=gvar84[:], func=AF.Ln, bias=eps8[:])
        nc.scalar.activation(out=grs[:, :, 0], in_=srt84[:], func=AF.Exp,
                             scale=-0.5)
        nc.vector.tensor_copy(out=grs[:, :, 1], in_=gmu84[:])
        psr = bsh.tile([128, NCO, 2], F32, tag="b", name="psr")
        nc.tensor.matmul(psr.rearrange("p co s -> p (co s)"), gmatT_sb[:],
                         grs.rearrange("g co s -> g (co s)"),
                         start=True, stop=True)
        nc.vector.tensor_copy(out=rg_pc[:], in_=psr[:])
        nc.vector.tensor_tensor(out=a_pc[:], in0=gnw_pc[:], in1=rg_pc[:, :, 0],
                                op=MUL)
        nc.vector.tensor_tensor(out=tmp_pc[:], in0=rg_pc[:, :, 1], in1=a_pc[:], op=MUL)
        nc.vector.tensor_tensor(out=d_pc[:], in0=gnb_pc[:], in1=tmp_pc[:], op=SUB)

        # qbias = wq_h @ d + bq ; wqs = wqT * a (columns scaled per channel)
        qb = bsh.tile([DH, 1], F32, tag="b", name="qb")
        for co in range(NCO):
            nc.tensor.matmul(qb[:], wqT_sb[:, co, :], d_pc[:, co:co + 1],
                             start=(co == 0), stop=(co == NCO - 1))
        nc.vector.tensor_tensor(out=qbias[:], in0=qb[:], in1=bq_sb[:], op=ADD)
        for co in range(NCO):
            nc.vector.tensor_scalar_mul(out=wqs_bf[:, co, :], in0=wqT_sb[:, co, :],
                                        scalar1=a_pc[:, co:co + 1])

        # ---------------- Q projection ----------------
        def q_proj(ic):
            sl = slice(ic * IC, (ic + 1) * IC)
            pq = bsh.tile([DH, IC], F32, tag="b", name="pq")
            for co in range(NCO):
                nc.tensor.matmul(pq[:], wqs_bf[:, co, :], xb_sb[:, co, sl],
                                 start=(co == 0), stop=(co == NCO - 1))
            with nc.allow_low_precision(reason="fp8 attention operand"):
                nc.vector.tensor_scalar(out=q8[:, sl], in0=pq[:],
                                        scalar1=qbias[:], scalar2=None, op0=ADD)

        # ---------------- attention blocks ----------------
        pav_tiles = {}

        def att_block(ic, qt):
            """QK + exp + AV for chunk ic over quarter qt's key tiles."""
            sl = slice(ic * IC, (ic + 1) * IC)
            if qt == 0:
                pav_tiles[ic] = psav.tile([96, IC], F32, tag="pav", name="pav")
            pav = pav_tiles[ic]
            for g in range(qt * (NG // 4), (qt + 1) * (NG // 4)):
                jA, jB = 2 * g, 2 * g + 1
                tt = pst.tile([128, 2, IC], F32, tag="tps", name="tps")
                nc.tensor.matmul(tt[:, 0, :],
                                 bc2(k8[:, jA * 128:(jA + 1) * 128], DH, 128),
                                 bc2(q8[:, sl], DH, IC),
                                 start=True, stop=True, perf_mode=DR)
                nc.tensor.matmul(tt[:, 1, :],
                                 bc2(k8[:, jB * 128:(jB + 1) * 128], DH, 128),
                                 bc2(q8[:, sl], DH, IC),
                                 start=True, stop=True, perf_mode=DR)
                ee = expp.tile([128, 2, IC], FP8, tag="exp", name="exp")
                if g in TRICK_GROUPS:
                    bf = bfp.tile([128, 2, IC], F32, tag="bf", name="bf")
                    nc.vector.tensor_scalar(out=bf[:], in0=tt[:],
                                            scalar1=TRICK_A, scalar2=TRICK_B,
                                            op0=MUL, op1=ADD)
                    with nc.allow_low_precision(reason="bit-trick exp"):
                        nc.gpsimd.tensor_copy(out=ee[:].bitcast(U8), in_=bf[:])
                else:
                    nc.scalar.activation(out=ee[:], in_=tt[:], func=AF.Exp,
                                         scale=EXP_SCALE)
                nc.tensor.matmul(pav[:], vT8[:, g, :, :], ee[:],
                                 start=(g == 0), stop=(g == NG - 1),
                                 perf_mode=DR)

        def att_finish(ic):
            """normalize + output projection + bias/residual + store."""
            sl = slice(ic * IC, (ic + 1) * IC)
            pav = pav_tiles.pop(ic)
            rden = attp.tile([1, IC], F32R, tag="rden", name="rden")
            with nc.allow_low_precision(reason="f32r matmul operand"):
                nc.vector.reciprocal(out=rden[:], in_=pav[64:65, :])
            rbp = bsh.tile([DH, IC], F32, tag="b", name="rbp")
            nc.tensor.matmul(rbp[:], ones_c0[:], rden[:], start=True, stop=True)
            rb = attp.tile([DH, IC], F32, tag="rb", name="rb")
            nc.vector.tensor_copy(out=rb[:], in_=rbp[:])
            attn8 = attp.tile([DH, IC], FP8, tag="attn", name="attn")
            with nc.allow_low_precision(reason="fp8 attention operand"):
                nc.vector.tensor_tensor(out=attn8[:], in0=pav[0:DH, :],
                                        in1=rb[:], op=MUL)
            for ot in range(NCO):
                po = bsh.tile([128, IC], F32, tag="b", name="po")
                nc.tensor.matmul(po[:],
                                 bc2(woT8_sb[:, ot * 128:(ot + 1) * 128], DH, 128),
                                 bc2(attn8[:], DH, IC),
                                 start=True, stop=True, perf_mode=DR)
                pt = outp.tile([128, IC], BF16, tag="pt", name="pt")
                with nc.allow_low_precision(reason="bf16 partial output"):
                    nc.vector.scalar_tensor_tensor(out=pt[:], in0=po[:],
                                                   scalar=O_SCALE,
                                                   in1=xm_sb[:, ot, sl],
                                                   op0=MUL, op1=ADD)
                nc.sync.dma_start(out=pvw[:, ot, sl], in_=pt[:])

        # ---------------- emission schedule ----------------
        q_proj(0)
        q_proj(1)
        for qt in range(4):
            att_block(0, qt)
        q_proj(2)
        for qt in range(4):
            att_block(1, qt)
        q_proj(3)
        nq = 4
        for ic in range(2, NIC):
            att_finish(ic - 2)
            if nq < NIC:
                q_proj(nq)
                nq += 1
            if ic < NIC - 1:
                for qt in range(4):
                    att_block(ic, qt)
            else:
                att_block(ic, 0)
                att_block(ic, 1)
                att_finish(ic - 1)  # chunk 6 finishes amid chunk 7's blocks
                att_block(ic, 2)
                att_block(ic, 3)
        att_finish(NIC - 1)

        for p in [stats_pool, outp, attp, bfp, expp, bsh, psav, pst, singles]:
            p.release()

    nc.compile()
    return nc


GMAT = (np.arange(128)[:, None] // 16 == np.arange(8)[None, :]).astype(np.float32)
GMATT = np.ascontiguousarray(GMAT.T)
NP8 = mybir.dt.np(FP8)
NPBF = mybir.dt.np(BF16)
VCOL = np.zeros((128, 32), dtype=np.float32)
VCOL[:, 0] = 1.0
VCOL = VCOL.astype(NP8)

_NC_CACHE = None


def get_nc():
    global _NC_CACHE
    if _NC_CACHE is None:
        _NC_CACHE = build_nc()
    return _NC_CACHE


def kernel(x, context, gn_w, gn_b, wq, bq, wk, bk, wv, bv, wo, bo):
    from concourse.bass_utils import run_bass_kernel_spmd

    x = np.asarray(x, dtype=np.float32)
    context = np.asarray(context, dtype=np.float32)
    gn_w = np.asarray(gn_w, dtype=np.float32)
    gn_b = np.asarray(gn_b, dtype=np.float32)
    wq = np.asarray(wq, dtype=np.float32)
    bq = np.asarray(bq, dtype=np.float32)
    wk = np.asarray(wk, dtype=np.float32)
    bk = np.asarray(bk, dtype=np.float32)
    wv = np.asarray(wv, dtype=np.float32)
    bv = np.asarray(bv, dtype=np.float32)
    wo = np.asarray(wo, dtype=np.float32)
    bo = np.asarray(bo, dtype=np.float32)

    B, C, H, W = x.shape
    xb2 = np.ascontiguousarray(x.reshape(C, H * W)).astype(NPBF)
    ctx82 = np.ascontiguousarray(context.reshape(CTXC, H * W)).astype(NP8)

    in_maps = []
    for h in range(NH):
        hs = slice(h * DH, (h + 1) * DH)
        in_maps.append({
            "xb": xb2,
            "ctx8": ctx82,
            "gn_w": gn_w,
            "gn_b": gn_b,
            "wqT": np.ascontiguousarray(wq[hs, :].T),
            "wkT8": np.ascontiguousarray(wk[hs, :].T * W8).astype(NP8),
            "wvT8": np.ascontiguousarray(wv[hs, :].T * W8).astype(NP8),
            "woT8": np.ascontiguousarray(wo[:, hs].T * W8).astype(NP8),
            "woT": np.ascontiguousarray(wo[:, hs].T),
            "bq": np.ascontiguousarray(bq[hs]),
            "bk8": np.ascontiguousarray(bk[hs] * W8),
            "bv": np.ascontiguousarray(bv[hs]),
            "bo": bo,
            "is0": np.array([[1.0 if h == 0 else 0.0]], dtype=np.float32),
            "gmat": GMAT,
            "gmatT": GMATT,
            "vcol": VCOL,
        })

    nc = get_nc()
    res = run_bass_kernel_spmd(nc, in_maps, core_ids=list(range(NH)))
    acc = np.zeros((C, H * W), dtype=np.float64)
    for h in range(NH):
        acc += res.results[h]["partial"].astype(np.float64)
    return acc.astype(np.float32).reshape(B, C, H, W)


# revision 28
# speedup vs baseline: 1.3154x; 1.3154x over previous
"""Trainium2 Bass kernel for a CrossAttentionBlock (GroupNorm + 8-head
cross-attention + output projection + residual).

Sharding: one attention head per NeuronCore (8 heads / 8 cores).  Each core
computes its head's partial output projection wo[:, h] @ attn_h; the host sums
the 8 partials (partial-sum unshard).  Residual and output bias are added on
core 0 only (mask input), so the host-side sum is a pure reduce.

v3: fp8e4 DoubleRow matmuls + multi-engine softmax exp.
 - x arrives bf16 (Q proj in bf16, GroupNorm stats, residual); context and
   attention weights arrive fp8e4, pre-scaled by 8 on the host so w*8 clears
   the e4m3 subnormal threshold.  Scale corrections fold into the exp scale
   and the output epilogue constant.
 - Scores tt = 2*k8^T q8 via stride-0 broadcast of the DoubleRow k-subtile
   dim; vT is computed directly as ctx8^T wv8 (keys on partitions) with a
   ones column riding the AV matmul as the softmax denominator (stationary
   width padded to 96 - DoubleRow needs a multiple of 32).
 - K/vT PSUM evacuation runs on ScalarE (activation Identity/Copy) so the
   whole K/V production overlaps the DVE GroupNorm-stats window; the
   residual+bias epilogue tiles (xm) are precomputed on GpSimd up front.
 - exp runs on ScalarE for 11/16 groups per chunk; the other 5 use a
   bit-trick path (DVE: b = tt*A + B in f32; GpSimd: f32->uint8 round, bits
   reinterpreted as e4m3 ~ exp) to spread the elementwise softmax cost over
   three engines.  The PWL error is a global constant times +-2.6% ripple;
   the constant cancels in softmax.

Self-contained: hardcodes all shapes from the problem spec.
"""

import math
import sys

sys.path.insert(0, "/opt/trn_rl_repo")

import numpy as np

import concourse.bass as bass
import concourse.tile as tile
from concourse import bacc, mybir

F32 = mybir.dt.float32
F32R = mybir.dt.float32r
BF16 = mybir.dt.bfloat16
FP8 = mybir.dt.float8e4
U8 = mybir.dt.uint8
DR = mybir.MatmulPerfMode.DoubleRow
AF = mybir.ActivationFunctionType

CH = 512          # x channels
CTXC = 768        # context channels
N = 4096          # spatial positions (64*64)
NH = 8            # heads
DH = 64           # head dim
G = 32            # groupnorm groups
EPS = 1e-5
NCO = CH // 128   # x channel blocks (4)
NCK = CTXC // 128  # ctx channel blocks (6)
IC = 512          # query-chunk size
NIC = N // IC     # 8 query chunks
NJT = N // 128    # 32 key tiles
NG = NJT // 2     # 16 exp groups (2 key tiles each) per chunk
SCALE = 1.0 / 8.0  # 1/sqrt(DH)
W8 = 8.0           # host-side fp8 weight pre-scale
# tt = 2 * (8 wk)^T q = 16 * (k.q); true score = SCALE * (k.q)
EXP_SCALE = SCALE / 16.0
# attn8 = C0 * pav[0:64] / den = C0 * 8 * sum(p v); attn8 ~ 1024 * sum(p v)
C0 = 128.0
# po = 2 * (8 wo)^T attn8 = 2*8*8*128 * o_true
O_SCALE = 1.0 / 16384.0
# bit-trick exp: uint8 bits of e4m3 ~ 8*(log2(v)+7); round mode on device
TRICK_A = EXP_SCALE * 8.0 / math.log(2.0)
TRICK_B = 55.8
# groups per chunk routed to the DVE+Pool bit-trick path
TRICK_GROUPS = frozenset({1, 4, 7, 10, 13})
# AV matmuls trail the QK/exp stream by this many groups so the in-order PE
# never stalls waiting for a fresh exp tile (the trick chain's DVE+Pool
# latency is ~3.5us, so the lag must cover ~5 groups of Act time)
AV_LAG = 5

ADD = mybir.AluOpType.add
SUB = mybir.AluOpType.subtract
MUL = mybir.AluOpType.mult


def bc2(ap, p, n):
    """Insert a stride-0 k-subtile dim of size 2 (DoubleRow dup trick)."""
    return ap.rearrange("p (o n) -> p o n", o=1).to_broadcast((p, 2, n))


def build_nc():
    nc = bacc.Bacc("TRN2", num_devices=8, debug=False)

    xb = nc.dram_tensor("xb", (CH, N), BF16, kind="ExternalInput")
    ctx8 = nc.dram_tensor("ctx8", (CTXC, N), FP8, kind="ExternalInput")
    gn_w = nc.dram_tensor("gn_w", (CH,), F32, kind="ExternalInput")
    gn_b = nc.dram_tensor("gn_b", (CH,), F32, kind="ExternalInput")
    wqT = nc.dram_tensor("wqT", (CH, DH), F32, kind="ExternalInput")
    wkT8 = nc.dram_tensor("wkT8", (CTXC, DH), FP8, kind="ExternalInput")
    wvT8 = nc.dram_tensor("wvT8", (CTXC, DH), FP8, kind="ExternalInput")
    woT8 = nc.dram_tensor("woT8", (DH, CH), FP8, kind="ExternalInput")
    woT = nc.dram_tensor("woT", (DH, CH), F32, kind="ExternalInput")
    bq = nc.dram_tensor("bq", (DH,), F32, kind="ExternalInput")
    bk8 = nc.dram_tensor("bk8", (DH,), F32, kind="ExternalInput")
    bv = nc.dram_tensor("bv", (DH,), F32, kind="ExternalInput")
    bo = nc.dram_tensor("bo", (CH,), F32, kind="ExternalInput")
    is0 = nc.dram_tensor("is0", (1, 1), F32, kind="ExternalInput")
    gmat = nc.dram_tensor("gmat", (128, 8), F32, kind="ExternalInput")
    gmatT = nc.dram_tensor("gmatT", (8, 128), F32, kind="ExternalInput")
    vcol = nc.dram_tensor("vcol", (128, 32), FP8, kind="ExternalInput")
    partial = nc.dram_tensor("partial", (CH, N), BF16, kind="ExternalOutput")

    pvw = partial.rearrange("(co p) n -> p co n", p=128)

    with tile.TileContext(nc) as tc:
        singles = tc.alloc_tile_pool(name="singles", bufs=1)
        # PSUM: pst 3x2 + psav 1 + bsh 1 = 8 banks.  Three score slots let the
        # QK matmuls run two groups ahead of the exp stream, hiding the
        # slot-recycle semaphore latency behind two full exp instructions.
        pst = tc.alloc_tile_pool(name="pst", bufs=3, space="PSUM")
        psav = tc.alloc_tile_pool(name="psav", bufs=1, space="PSUM")
        bsh = tc.alloc_tile_pool(name="bsh", bufs=1, space="PSUM")
        expp = tc.alloc_tile_pool(name="expp", bufs=10)
        bfp = tc.alloc_tile_pool(name="bfp", bufs=4)
        attp = tc.alloc_tile_pool(name="attp", bufs=2)
        outp = tc.alloc_tile_pool(name="outp", bufs=2)
        stats_pool = tc.alloc_tile_pool(name="stats", bufs=2)

        # ---------------- persistent SBUF tiles ----------------
        xb_sb = singles.tile([128, NCO, N], BF16)
        ctx8_sb = singles.tile([128, NCK, N], FP8)
        xm_sb = singles.tile([128, NCO, N], F32)
        q8 = singles.tile([DH, N], FP8)
        k8 = singles.tile([DH, N], FP8)
        vT8 = singles.tile([128, NG, 2, 96], FP8)
        wqT_sb = singles.tile([128, NCO, DH], F32)
        wqs_bf = singles.tile([128, NCO, DH], BF16)
        wkT8_sb = singles.tile([128, NCK, DH], FP8)
        wvT8_sb = singles.tile([128, NCK, DH], FP8)
        woT8_sb = singles.tile([DH, CH], FP8)
        woT_sb = singles.tile([DH, CH], F32)
        ones_c0 = singles.tile([1, DH], F32R)
        ones_f = singles.tile([1, DH], F32)
        gmat_sb = singles.tile([128, 8], F32)
        gmatT_sb = singles.tile([8, 128], F32)
        gnw_pc = singles.tile([128, NCO], F32)
        gnb_pc = singles.tile([128, NCO], F32)
        mvall = singles.tile([128, NCO, 2], F32)
        mv3 = singles.tile([128, NCO, 3], F32)
        gsm = singles.tile([8, NCO, 3], F32)
        gmu84 = singles.tile([8, NCO], F32)
        gvar84 = singles.tile([8, NCO], F32)
        srt84 = singles.tile([8, NCO], F32)
        grs = singles.tile([8, NCO, 2], F32)
        rg_pc = singles.tile([128, NCO, 2], F32)
        tmp_pc = singles.tile([128, NCO], F32)
        eps8 = singles.tile([8, 1], F32)
        a_pc = singles.tile([128, NCO], F32)
        d_pc = singles.tile([128, NCO], F32)
        qbias = singles.tile([DH, 1], F32)
        bq_sb = singles.tile([DH, 1], F32)
        kb_sb = singles.tile([DH, 1], F32)
        bv64 = singles.tile([DH, 1], F32)
        wobv_pc = singles.tile([128, NCO], F32)
        bomv = singles.tile([128, NCO], F32)
        bo_pc = singles.tile([128, NCO], F32)
        bom = singles.tile([128, NCO], F32)
        msk = singles.tile([128, 1], F32)
        expwarm = singles.tile([1, 1], F32)

        # ---------------- input loads ----------------
        # DMA issue on SP is sequential (~650ns each): x blocks first (the
        # GroupNorm stats path is the startup critical path), then the small
        # weights, then context as 6 whole-row transfers.
        xv = xb.rearrange("(co p) n -> p co n", p=128)
        cv = ctx8.rearrange("(ck p) n -> p ck n", p=128)
        for co in range(NCO):
            nc.sync.dma_start(out=xb_sb[:, co, :], in_=xv[:, co, :])
        nc.sync.dma_start(out=wqT_sb[:], in_=wqT.rearrange("(co p) d -> p co d", p=128))
        nc.sync.dma_start(out=wkT8_sb[:], in_=wkT8.rearrange("(ck p) d -> p ck d", p=128))
        nc.sync.dma_start(out=wvT8_sb[:], in_=wvT8.rearrange("(ck p) d -> p ck d", p=128))
        nc.sync.dma_start(out=woT8_sb[:], in_=woT8[:])
        nc.sync.dma_start(out=woT_sb[:], in_=woT[:])
        nc.sync.dma_start(out=gnw_pc[:], in_=gn_w.rearrange("(co p) -> p co", p=128))
        nc.sync.dma_start(out=gnb_pc[:], in_=gn_b.rearrange("(co p) -> p co", p=128))
        nc.sync.dma_start(out=gmat_sb[:], in_=gmat[:])
        nc.sync.dma_start(out=gmatT_sb[:], in_=gmatT[:])
        nc.sync.dma_start(out=bq_sb[:], in_=bq[:][:, None])
        nc.sync.dma_start(out=kb_sb[:], in_=bk8[:][:, None])
        nc.sync.dma_start(out=bv64[:], in_=bv[:][:, None])
        nc.sync.dma_start(out=bo_pc[:], in_=bo.rearrange("(co p) -> p co", p=128))
        nc.sync.dma_start(out=msk[:], in_=is0[:].to_broadcast((128, 1)))
        # vT pad columns 64:96 = [1, 0, ..., 0] via broadcast DMA
        nc.sync.dma_start(
            out=vT8[:, :, :, 64:96].rearrange("p a b n -> p (a b) n"),
            in_=vcol.rearrange("p (a n) -> p a n", a=1).to_broadcast(
                (128, NG * 2, 32)))
        for ck in range(NCK):
            nc.sync.dma_start(out=ctx8_sb[:, ck, :], in_=cv[:, ck, :])

        # small constants + activation table warm (Ln+Exp share a set)
        nc.vector.memset(ones_f[:], C0)
        nc.vector.tensor_copy(out=ones_c0[:], in_=ones_f[:])
        nc.vector.memset(eps8[:], EPS)
        nc.vector.memset(expwarm[:], 1.0)
        nc.scalar.activation(out=expwarm[:], in_=expwarm[:], func=AF.Ln)
        nc.scalar.activation(out=expwarm[:], in_=expwarm[:], func=AF.Exp)

        # output-bias terms: bomv = bo*msk + wo@bv
        nc.vector.tensor_scalar_mul(out=bom[:], in0=bo_pc[:], scalar1=msk[:])
        for ot in range(NCO):
            pwb = bsh.tile([128, 1], F32, tag="b", name="pwb")
            nc.tensor.matmul(pwb[:], woT_sb[:, ot * 128:(ot + 1) * 128], bv64[:],
                             start=True, stop=True)
            nc.vector.tensor_copy(out=wobv_pc[:, ot:ot + 1], in_=pwb[:])
        nc.vector.tensor_tensor(out=bomv[:], in0=bom[:], in1=wobv_pc[:], op=ADD)

        # ---------------- K and vT production (PE + ScalarE) ----------------
        def kv_quarter(qt):
            for lc in range(2):
                ic = qt * 2 + lc
                sl = slice(ic * IC, (ic + 1) * IC)
                pk = bsh.tile([DH, IC], F32, tag="b", name="pk")
                for j in range(NCK // 2):
                    nc.tensor.matmul(pk[:], wkT8_sb[:, 2 * j:2 * j + 2, :],
                                     ctx8_sb[:, 2 * j:2 * j + 2, sl],
                                     start=(j == 0), stop=(j == NCK // 2 - 1),
                                     perf_mode=DR)
                with nc.allow_low_precision(reason="fp8 attention operand"):
                    nc.scalar.activation(out=k8[:, sl], in_=pk[:],
                                         func=AF.Identity, bias=kb_sb[:])
            for h in range(2):  # two 4-key-tile batches per quarter
                kt0 = qt * 8 + h * 4
                pvt = bsh.tile([128, 4, DH], F32, tag="b", name="pvt")
                for i in range(4):
                    ks = slice((kt0 + i) * 128, (kt0 + i + 1) * 128)
                    for j in range(NCK // 2):
                        nc.tensor.matmul(pvt[:, i, :],
                                         ctx8_sb[:, 2 * j:2 * j + 2, ks],
                                         wvT8_sb[:, 2 * j:2 * j + 2, :],
                                         start=(j == 0),
                                         stop=(j == NCK // 2 - 1),
                                         perf_mode=DR)
                dst = vT8[:, kt0 // 2:kt0 // 2 + 2, :, 0:DH].rearrange(
                    "p a b d -> p (a b) d")
                with nc.allow_low_precision(reason="fp8 attention operand"):
                    nc.scalar.activation(out=dst, in_=pvt[:], func=AF.Copy)

        for qt in range(4):
            kv_quarter(qt)

        # residual epilogue tiles, precomputed on GpSimd during startup
        for co in range(NCO):
            nc.gpsimd.tensor_scalar(out=xm_sb[:, co, :], in0=xb_sb[:, co, :],
                                    scalar1=msk[:], scalar2=bomv[:, co:co + 1],
                                    op0=MUL, op1=ADD)

        # ---------------- groupnorm statistics (DVE) ----------------
        for co in range(NCO):
            st = stats_pool.tile([128, 8, 6], F32)
            xvw = xb_sb[:, co, :].rearrange("p (s c) -> p s c", c=512)
            for s in range(8):
                nc.vector.bn_stats(out=st[:, s, :], in_=xvw[:, s, :])
            nc.vector.bn_aggr(out=mvall[:, co, :], in_=st[:])
        # group stats via tiny PE matmuls, batched over channel blocks
        nc.vector.tensor_copy(out=mv3[:, :, 0:2], in_=mvall[:])
        nc.vector.tensor_tensor(out=mv3[:, :, 2:3], in0=mvall[:, :, 0:1],
                                in1=mvall[:, :, 0:1], op=MUL)
        psg = bsh.tile([8, NCO, 3], F32, tag="b", name="psg")
        nc.tensor.matmul(psg.rearrange("g co s -> g (co s)"), gmat_sb[:],
                         mv3.rearrange("p co s -> p (co s)"),
                         start=True, stop=True)
        nc.vector.tensor_copy(out=gsm[:], in_=psg[:])
        nc.vector.tensor_scalar_mul(out=gmu84[:], in0=gsm[:, :, 0], scalar1=1.0 / 16.0)
        nc.vector.tensor_tensor(out=gvar84[:], in0=gsm[:, :, 1], in1=gsm[:, :, 2],
                                op=ADD)
        nc.vector.tensor_scalar_mul(out=gvar84[:], in0=gvar84[:], scalar1=1.0 / 16.0)
        nc.vector.tensor_tensor(out=srt84[:], in0=gmu84[:], in1=gmu84[:], op=MUL)
        nc.vector.tensor_tensor(out=gvar84[:], in0=gvar84[:], in1=srt84[:], op=SUB)
        # rstd = exp(-0.5*ln(var+eps)): stays on the Ln+Exp activation set
        nc.scalar.activation(out=srt84[:], in_=gvar84[:], func=AF.Ln, bias=eps8[:])
        nc.scalar.activation(out=grs[:, :, 0], in_=srt84[:], func=AF.Exp,
                             scale=-0.5)
        nc.vector.tensor_copy(out=grs[:, :, 1], in_=gmu84[:])
        psr = bsh.tile([128, NCO, 2], F32, tag="b", name="psr")
        nc.tensor.matmul(psr.rearrange("p co s -> p (co s)"), gmatT_sb[:],
                         grs.rearrange("g co s -> g (co s)"),
                         start=True, stop=True)
        nc.vector.tensor_copy(out=rg_pc[:], in_=psr[:])
        nc.vector.tensor_tensor(out=a_pc[:], in0=gnw_pc[:], in1=rg_pc[:, :, 0],
                                op=MUL)
        nc.vector.tensor_tensor(out=tmp_pc[:], in0=rg_pc[:, :, 1], in1=a_pc[:], op=MUL)
        nc.vector.tensor_tensor(out=d_pc[:], in0=gnb_pc[:], in1=tmp_pc[:], op=SUB)

        # qbias = wq_h @ d + bq ; wqs = wqT * a (columns scaled per channel)
        qb = bsh.tile([DH, 1], F32, tag="b", name="qb")
        for co in range(NCO):
            nc.tensor.matmul(qb[:], wqT_sb[:, co, :], d_pc[:, co:co + 1],
                             start=(co == 0), stop=(co == NCO - 1))
        nc.vector.tensor_tensor(out=qbias[:], in0=qb[:], in1=bq_sb[:], op=ADD)
        for co in range(NCO):
            nc.vector.tensor_scalar_mul(out=wqs_bf[:, co, :], in0=wqT_sb[:, co, :],
                                        scalar1=a_pc[:, co:co + 1])

        # ---------------- Q projection ----------------
        def q_proj(ic):
            sl = slice(ic * IC, (ic + 1) * IC)
            pq = bsh.tile([DH, IC], F32, tag="b", name="pq")
            for co in range(NCO):
                nc.tensor.matmul(pq[:], wqs_bf[:, co, :], xb_sb[:, co, sl],
                                 start=(co == 0), stop=(co == NCO - 1))
            with nc.allow_low_precision(reason="fp8 attention operand"):
                nc.vector.tensor_scalar(out=q8[:, sl], in0=pq[:],
                                        scalar1=qbias[:], scalar2=None, op0=ADD)

        # ---------------- attention blocks ----------------
        pav_tiles = {}
        av_pending = []
        deferred = []  # finish-epilogue pieces, pumped between groups

        def pump():
            if deferred:
                deferred.pop(0)()

        def emit_av(pav, g, ee):
            nc.tensor.matmul(pav[:], vT8[:, g, :, :], ee[:],
                             start=(g == 0), stop=(g == NG - 1),
                             perf_mode=DR)

        def att_block(ic, qt):
            """QK + exp for chunk ic over quarter qt's key tiles; the AV
            matmuls trail by AV_LAG groups (drained at qt==3)."""
            sl = slice(ic * IC, (ic + 1) * IC)
            if qt == 0:
                pav_tiles[ic] = psav.tile([96, IC], F32, tag="pav", name="pav")
            pav = pav_tiles[ic]
            for g in range(qt * (NG // 4), (qt + 1) * (NG // 4)):
                pump()
                jA, jB = 2 * g, 2 * g + 1
                tt = pst.tile([128, 2, IC], F32, tag="tps", name="tps")
                nc.tensor.matmul(tt[:, 0, :],
                                 bc2(k8[:, jA * 128:(jA + 1) * 128], DH, 128),
                                 bc2(q8[:, sl], DH, IC),
                                 start=True, stop=True, perf_mode=DR)
                nc.tensor.matmul(tt[:, 1, :],
                                 bc2(k8[:, jB * 128:(jB + 1) * 128], DH, 128),
                                 bc2(q8[:, sl], DH, IC),
                                 start=True, stop=True, perf_mode=DR)
                ee = expp.tile([128, 2, IC], FP8, tag="exp", name="exp")
                if g in TRICK_GROUPS:
                    bf = bfp.tile([128, 2, IC], F32, tag="bf", name="bf")
                    nc.vector.tensor_scalar(out=bf[:], in0=tt[:],
                                            scalar1=TRICK_A, scalar2=TRICK_B,
                                            op0=MUL, op1=ADD)
                    with nc.allow_low_precision(reason="bit-trick exp"):
                        nc.gpsimd.tensor_copy(out=ee[:].bitcast(U8), in_=bf[:])
                else:
                    nc.scalar.activation(out=ee[:], in_=tt[:], func=AF.Exp,
                                         scale=EXP_SCALE)
                av_pending.append((pav, g, ee))
                if len(av_pending) > AV_LAG:
                    emit_av(*av_pending.pop(0))
            if qt == 3:
                while av_pending:
                    emit_av(*av_pending.pop(0))

        def att_finish_head(ic):
            """normalize + attn8; the per-channel-block output projection
            pieces are queued on `deferred` and pumped between later groups
            so the DVE work spreads out instead of bursting."""
            sl = slice(ic * IC, (ic + 1) * IC)
            pav = pav_tiles.pop(ic)
            rden = attp.tile([1, IC], F32R, tag="rden", name="rden")
            with nc.allow_low_precision(reason="f32r matmul operand"):
                nc.vector.reciprocal(out=rden[:], in_=pav[64:65, :])
            rbp = bsh.tile([DH, IC], F32, tag="b", name="rbp")
            nc.tensor.matmul(rbp[:], ones_c0[:], rden[:], start=True, stop=True)
            rb = attp.tile([DH, IC], F32, tag="rb", name="rb")
            nc.vector.tensor_copy(out=rb[:], in_=rbp[:])
            attn8 = attp.tile([DH, IC], FP8, tag="attn", name="attn")
            with nc.allow_low_precision(reason="fp8 attention operand"):
                nc.vector.tensor_tensor(out=attn8[:], in0=pav[0:DH, :],
                                        in1=rb[:], op=MUL)

            def piece(ot, attn8=attn8, sl=sl):
                po = bsh.tile([128, IC], F32, tag="b", name="po")
                nc.tensor.matmul(po[:],
                                 bc2(woT8_sb[:, ot * 128:(ot + 1) * 128], DH, 128),
                                 bc2(attn8[:], DH, IC),
                                 start=True, stop=True, perf_mode=DR)
                pt = outp.tile([128, IC], BF16, tag="pt", name="pt")
                with nc.allow_low_precision(reason="bf16 partial output"):
                    nc.vector.scalar_tensor_tensor(out=pt[:], in0=po[:],
                                                   scalar=O_SCALE,
                                                   in1=xm_sb[:, ot, sl],
                                                   op0=MUL, op1=ADD)
                nc.sync.dma_start(out=pvw[:, ot, sl], in_=pt[:])

            for ot in range(NCO):
                deferred.append(lambda ot=ot: piece(ot))

        # ---------------- emission schedule ----------------
        q_proj(0)
        q_proj(1)
        for qt in range(4):
            att_block(0, qt)
        q_proj(2)
        for qt in range(4):
            att_block(1, qt)
        q_proj(3)
        nq = 4
        for ic in range(2, NIC):
            att_finish_head(ic - 2)
            if nq < NIC:
                q_proj(nq)
                nq += 1
            if ic < NIC - 1:
                for qt in range(4):
                    att_block(ic, qt)
            else:
                att_block(ic, 0)
                att_block(ic, 1)
                att_finish_head(ic - 1)  # chunk 6 finishes amid chunk 7
                att_block(ic, 2)
                att_block(ic, 3)
        while deferred:
            pump()
        att_finish_head(NIC - 1)
        while deferred:
            pump()

        for p in [stats_pool, outp, attp, bfp, expp, bsh, psav, pst, singles]:
            p.release()

    nc.compile()
    return nc


GMAT = (np.arange(128)[:, None] // 16 == np.arange(8)[None, :]).astype(np.float32)
GMATT = np.ascontiguousarray(GMAT.T)
NP8 = mybir.dt.np(FP8)
NPBF = mybir.dt.np(BF16)
VCOL = np.zeros((128, 32), dtype=np.float32)
VCOL[:, 0] = 1.0
VCOL = VCOL.astype(NP8)

_NC_CACHE = None


def get_nc():
    global _NC_CACHE
    if _NC_CACHE is None:
        _NC_CACHE = build_nc()
    return _NC_CACHE


def kernel(x, context, gn_w, gn_b, wq, bq, wk, bk, wv, bv, wo, bo):
    from concourse.bass_utils import run_bass_kernel_spmd

    x = np.asarray(x, dtype=np.float32)
    context = np.asarray(context, dtype=np.float32)
    gn_w = np.asarray(gn_w, dtype=np.float32)
    gn_b = np.asarray(gn_b, dtype=np.float32)
    wq = np.asarray(wq, dtype=np.float32)
    bq = np.asarray(bq, dtype=np.float32)
    wk = np.asarray(wk, dtype=np.float32)
    bk = np.asarray(bk, dtype=np.float32)
    wv = np.asarray(wv, dtype=np.float32)
    bv = np.asarray(bv, dtype=np.float32)
    wo = np.asarray(wo, dtype=np.float32)
    bo = np.asarray(bo, dtype=np.float32)

    B, C, H, W = x.shape
    xb2 = np.ascontiguousarray(x.reshape(C, H * W)).astype(NPBF)
    ctx82 = np.ascontiguousarray(context.reshape(CTXC, H * W)).astype(NP8)

    in_maps = []
    for h in range(NH):
        hs = slice(h * DH, (h + 1) * DH)
        in_maps.append({
            "xb": xb2,
            "ctx8": ctx82,
            "gn_w": gn_w,
            "gn_b": gn_b,
            "wqT": np.ascontiguousarray(wq[hs, :].T),
            "wkT8": np.ascontiguousarray(wk[hs, :].T * W8).astype(NP8),
            "wvT8": np.ascontiguousarray(wv[hs, :].T * W8).astype(NP8),
            "woT8": np.ascontiguousarray(wo[:, hs].T * W8).astype(NP8),
            "woT": np.ascontiguousarray(wo[:, hs].T),
            "bq": np.ascontiguousarray(bq[hs]),
            "bk8": np.ascontiguousarray(bk[hs] * W8),
            "bv": np.ascontiguousarray(bv[hs]),
            "bo": bo,
            "is0": np.array([[1.0 if h == 0 else 0.0]], dtype=np.float32),
            "gmat": GMAT,
            "gmatT": GMATT,
            "vcol": VCOL,
        })

    nc = get_nc()
    res = run_bass_kernel_spmd(nc, in_maps, core_ids=list(range(NH)))
    acc = np.zeros((C, H * W), dtype=np.float64)
    for h in range(NH):
        acc += res.results[h]["partial"].astype(np.float64)
    return acc.astype(np.float32).reshape(B, C, H, W)


# revision 34
# speedup vs baseline: 1.3401x; 1.0188x over previous
"""Trainium2 Bass kernel for a CrossAttentionBlock (GroupNorm + 8-head
cross-attention + output projection + residual).

Sharding: one attention head per NeuronCore (8 heads / 8 cores).  Each core
computes its head's partial output projection wo[:, h] @ attn_h; the host sums
the 8 partials (partial-sum unshard).  Residual and output bias are added on
core 0 only (mask input), so the host-side sum is a pure reduce.

v3: fp8e4 DoubleRow matmuls + multi-engine softmax exp.
 - x arrives bf16 (Q proj in bf16, GroupNorm stats, residual); context and
   attention weights arrive fp8e4, pre-scaled by 8 on the host so w*8 clears
   the e4m3 subnormal threshold.  Scale corrections fold into the exp scale
   and the output epilogue constant.
 - Scores tt = 2*k8^T q8 via stride-0 broadcast of the DoubleRow k-subtile
   dim; vT is computed directly as ctx8^T wv8 (keys on partitions) with a
   ones column riding the AV matmul as the softmax denominator (stationary
   width padded to 96 - DoubleRow needs a multiple of 32).
 - K/vT PSUM evacuation runs on ScalarE (activation Identity/Copy) so the
   whole K/V production overlaps the DVE GroupNorm-stats window; the
   residual+bias epilogue tiles (xm) are precomputed on GpSimd up front.
 - exp runs on ScalarE for 11/16 groups per chunk; the other 5 use a
   bit-trick path (DVE: b = tt*A + B in f32; GpSimd: f32->uint8 round, bits
   reinterpreted as e4m3 ~ exp) to spread the elementwise softmax cost over
   three engines.  The PWL error is a global constant times +-2.6% ripple;
   the constant cancels in softmax.

Self-contained: hardcodes all shapes from the problem spec.
"""

import math
import sys

sys.path.insert(0, "/opt/trn_rl_repo")

import numpy as np

import concourse.bass as bass
import concourse.tile as tile
from concourse import bacc, mybir

F32 = mybir.dt.float32
F32R = mybir.dt.float32r
BF16 = mybir.dt.bfloat16
FP8 = mybir.dt.float8e4
U8 = mybir.dt.uint8
DR = mybir.MatmulPerfMode.DoubleRow
AF = mybir.ActivationFunctionType

CH = 512          # x channels
CTXC = 768        # context channels
N = 4096          # spatial positions (64*64)
NH = 8            # heads
DH = 64           # head dim
G = 32            # groupnorm groups
EPS = 1e-5
NCO = CH // 128   # x channel blocks (4)
NCK = CTXC // 128  # ctx channel blocks (6)
IC = 512          # query-chunk size
NIC = N // IC     # 8 query chunks
NJT = N // 128    # 32 key tiles
NG = NJT // 2     # 16 exp groups (2 key tiles each) per chunk
SCALE = 1.0 / 8.0  # 1/sqrt(DH)
W8 = 8.0           # host-side fp8 weight pre-scale
# tt = 2 * (8 wk)^T q = 16 * (k.q); true score = SCALE * (k.q)
EXP_SCALE = SCALE / 16.0
# attn8 = C0 * pav[0:64] / den = C0 * 8 * sum(p v); attn8 ~ 1024 * sum(p v)
C0 = 128.0
# po = 2 * (8 wo)^T attn8 = 2*8*8*128 * o_true
O_SCALE = 1.0 / 16384.0
# bit-trick exp: uint8 bits of e4m3 ~ 8*(log2(v)+7); round mode on device
TRICK_A = EXP_SCALE * 8.0 / math.log(2.0)
TRICK_B = 55.8
# groups per chunk routed to the DVE+Pool bit-trick path
TRICK_GROUPS = frozenset({1, 4, 7, 10, 13})
# AV matmuls trail the QK/exp stream by this many groups so the in-order PE
# never stalls waiting for a fresh exp tile (the trick chain's DVE+Pool
# latency is ~3.5us, so the lag must cover ~5 groups of Act time)
AV_LAG = 5

ADD = mybir.AluOpType.add
SUB = mybir.AluOpType.subtract
MUL = mybir.AluOpType.mult


def bc2(ap, p, n):
    """Insert a stride-0 k-subtile dim of size 2 (DoubleRow dup trick)."""
    return ap.rearrange("p (o n) -> p o n", o=1).to_broadcast((p, 2, n))


def build_nc():
    nc = bacc.Bacc("TRN2", num_devices=8, debug=False)

    xb = nc.dram_tensor("xb", (CH, N), BF16, kind="ExternalInput")
    ctx8 = nc.dram_tensor("ctx8", (CTXC, N), FP8, kind="ExternalInput")
    wqT = nc.dram_tensor("wqT", (CH, DH), F32, kind="ExternalInput")
    wkT8 = nc.dram_tensor("wkT8", (CTXC, DH), FP8, kind="ExternalInput")
    wvT8 = nc.dram_tensor("wvT8", (CTXC, DH), FP8, kind="ExternalInput")
    woT8 = nc.dram_tensor("woT8", (DH, CH), FP8, kind="ExternalInput")
    woT = nc.dram_tensor("woT", (DH, CH), F32, kind="ExternalInput")
    # packA: gnw(0:4) gnb(4:8) bo(8:12) gmat(12:20) msk(20:21), per-channel
    # row layout; packB: bq|bk8|bv columns
    packA = nc.dram_tensor("packA", (128, 21), F32, kind="ExternalInput")
    packB = nc.dram_tensor("packB", (DH, 3), F32, kind="ExternalInput")
    gmatT = nc.dram_tensor("gmatT", (8, 128), F32, kind="ExternalInput")
    vcol = nc.dram_tensor("vcol", (128, 32), FP8, kind="ExternalInput")
    partial = nc.dram_tensor("partial", (CH, N), BF16, kind="ExternalOutput")

    pvw = partial.rearrange("(co p) n -> p co n", p=128)

    with tile.TileContext(nc) as tc:
        singles = tc.alloc_tile_pool(name="singles", bufs=1)
        # PSUM: pst 3x2 + psav 1 + bsh 1 = 8 banks.  Three score slots let the
        # QK matmuls run two groups ahead of the exp stream, hiding the
        # slot-recycle semaphore latency behind two full exp instructions.
        pst = tc.alloc_tile_pool(name="pst", bufs=3, space="PSUM")
        psav = tc.alloc_tile_pool(name="psav", bufs=1, space="PSUM")
        bsh = tc.alloc_tile_pool(name="bsh", bufs=1, space="PSUM")
        expp = tc.alloc_tile_pool(name="expp", bufs=10)
        bfp = tc.alloc_tile_pool(name="bfp", bufs=4)
        attp = tc.alloc_tile_pool(name="attp", bufs=2)
        outp = tc.alloc_tile_pool(name="outp", bufs=2)
        stats_pool = tc.alloc_tile_pool(name="stats", bufs=2)

        # ---------------- persistent SBUF tiles ----------------
        xb_sb = singles.tile([128, NCO, N], BF16)
        ctx8_sb = singles.tile([128, NCK, N], FP8)
        xm_sb = singles.tile([128, NCO, N], F32)
        q8 = singles.tile([DH, N], FP8)
        k8 = singles.tile([DH, N], FP8)
        vT8 = singles.tile([128, NG, 2, 96], FP8)
        wqT_sb = singles.tile([128, NCO, DH], F32)
        wqs_bf = singles.tile([128, NCO, DH], BF16)
        wkT8_sb = singles.tile([128, NCK, DH], FP8)
        wvT8_sb = singles.tile([128, NCK, DH], FP8)
        woT8_sb = singles.tile([DH, CH], FP8)
        woT_sb = singles.tile([DH, CH], F32)
        ones_c0 = singles.tile([1, DH], F32R)
        ones_f = singles.tile([1, DH], F32)
        packA_sb = singles.tile([128, 21], F32)
        packB_sb = singles.tile([DH, 3], F32)
        gmatT_sb = singles.tile([8, 128], F32)
        mvall = singles.tile([128, NCO, 2], F32)
        mv3 = singles.tile([128, NCO, 3], F32)
        gsm = singles.tile([8, NCO, 3], F32)
        gmu84 = singles.tile([8, NCO], F32)
        gvar84 = singles.tile([8, NCO], F32)
        srt84 = singles.tile([8, NCO], F32)
        grs = singles.tile([8, NCO, 2], F32)
        rg_pc = singles.tile([128, NCO, 2], F32)
        tmp_pc = singles.tile([128, NCO], F32)
        eps8 = singles.tile([8, 1], F32)
        a_pc = singles.tile([128, NCO], F32)
        d_pc = singles.tile([128, NCO], F32)
        qbias = singles.tile([DH, 1], F32)
        wobv_pc = singles.tile([128, NCO], F32)
        bomv = singles.tile([128, NCO], F32)
        bom = singles.tile([128, NCO], F32)
        expwarm = singles.tile([1, 1], F32)
        # views into the packed small-constants tiles
        gnw_pc = packA_sb[:, 0:4]
        gnb_pc = packA_sb[:, 4:8]
        bo_pc = packA_sb[:, 8:12]
        gmat_sb = packA_sb[:, 12:20]
        msk = packA_sb[:, 20:21]
        bq_sb = packB_sb[:, 0:1]
        kb_sb = packB_sb[:, 1:2]
        bv64 = packB_sb[:, 2:3]

        # ---------------- input loads ----------------
        # DMA issue on SP is sequential (~650ns each): small packs first so
        # the bias chain unblocks, then x blocks (GroupNorm stats path), the
        # K/V weights + context (K/vT production), then the rest.
        xv = xb.rearrange("(co p) n -> p co n", p=128)
        cv = ctx8.rearrange("(ck p) n -> p ck n", p=128)
        nc.sync.dma_start(out=packA_sb[:], in_=packA[:])
        nc.sync.dma_start(out=packB_sb[:], in_=packB[:])
        for co in range(NCO):
            nc.sync.dma_start(out=xb_sb[:, co, :], in_=xv[:, co, :])
        nc.sync.dma_start(out=wkT8_sb[:], in_=wkT8.rearrange("(ck p) d -> p ck d", p=128))
        nc.sync.dma_start(out=wvT8_sb[:], in_=wvT8.rearrange("(ck p) d -> p ck d", p=128))
        for ck in range(NCK):
            nc.sync.dma_start(out=ctx8_sb[:, ck, :], in_=cv[:, ck, :])
        nc.sync.dma_start(out=wqT_sb[:], in_=wqT.rearrange("(co p) d -> p co d", p=128))
        nc.sync.dma_start(out=woT8_sb[:], in_=woT8[:])
        nc.sync.dma_start(out=woT_sb[:], in_=woT[:])
        nc.sync.dma_start(out=gmatT_sb[:], in_=gmatT[:])
        # vT pad columns 64:96 = [1, 0, ..., 0] via broadcast DMA
        nc.sync.dma_start(
            out=vT8[:, :, :, 64:96].rearrange("p a b n -> p (a b) n"),
            in_=vcol.rearrange("p (a n) -> p a n", a=1).to_broadcast(
                (128, NG * 2, 32)))

        # small constants + activation table warm (Ln+Exp share a set)
        nc.vector.memset(ones_f[:], C0)
        nc.vector.tensor_copy(out=ones_c0[:], in_=ones_f[:])
        nc.vector.memset(eps8[:], EPS)
        nc.vector.memset(expwarm[:], 1.0)
        nc.scalar.activation(out=expwarm[:], in_=expwarm[:], func=AF.Ln)
        nc.scalar.activation(out=expwarm[:], in_=expwarm[:], func=AF.Exp)

        # output-bias terms: bomv = bo*msk + wo@bv
        nc.vector.tensor_scalar_mul(out=bom[:], in0=bo_pc, scalar1=msk)
        for ot in range(NCO):
            pwb = bsh.tile([128, 1], F32, tag="b", name="pwb")
            nc.tensor.matmul(pwb[:], woT_sb[:, ot * 128:(ot + 1) * 128], bv64,
                             start=True, stop=True)
            nc.vector.tensor_copy(out=wobv_pc[:, ot:ot + 1], in_=pwb[:])
        nc.vector.tensor_tensor(out=bomv[:], in0=bom[:], in1=wobv_pc[:], op=ADD)

        # ---------------- K and vT production (PE + ScalarE) ----------------
        def kv_quarter(qt):
            for lc in range(2):
                ic = qt * 2 + lc
                sl = slice(ic * IC, (ic + 1) * IC)
                pk = bsh.tile([DH, IC], F32, tag="b", name="pk")
                for j in range(NCK // 2):
                    nc.tensor.matmul(pk[:], wkT8_sb[:, 2 * j:2 * j + 2, :],
                                     ctx8_sb[:, 2 * j:2 * j + 2, sl],
                                     start=(j == 0), stop=(j == NCK // 2 - 1),
                                     perf_mode=DR)
                with nc.allow_low_precision(reason="fp8 attention operand"):
                    nc.scalar.activation(out=k8[:, sl], in_=pk[:],
                                         func=AF.Identity, bias=kb_sb)
            for h in range(2):  # two 4-key-tile batches per quarter
                kt0 = qt * 8 + h * 4
                pvt = bsh.tile([128, 4, DH], F32, tag="b", name="pvt")
                for i in range(4):
                    ks = slice((kt0 + i) * 128, (kt0 + i + 1) * 128)
                    for j in range(NCK // 2):
                        nc.tensor.matmul(pvt[:, i, :],
                                         ctx8_sb[:, 2 * j:2 * j + 2, ks],
                                         wvT8_sb[:, 2 * j:2 * j + 2, :],
                                         start=(j == 0),
                                         stop=(j == NCK // 2 - 1),
                                         perf_mode=DR)
                dst = vT8[:, kt0 // 2:kt0 // 2 + 2, :, 0:DH].rearrange(
                    "p a b d -> p (a b) d")
                with nc.allow_low_precision(reason="fp8 attention operand"):
                    nc.scalar.activation(out=dst, in_=pvt[:], func=AF.Copy)

        for qt in range(4):
            kv_quarter(qt)

        # residual epilogue tiles, precomputed on GpSimd during startup
        for co in range(NCO):
            nc.gpsimd.tensor_scalar(out=xm_sb[:, co, :], in0=xb_sb[:, co, :],
                                    scalar1=msk, scalar2=bomv[:, co:co + 1],
                                    op0=MUL, op1=ADD)

        # ---------------- groupnorm statistics (DVE) ----------------
        for co in range(NCO):
            st = stats_pool.tile([128, 8, 6], F32)
            xvw = xb_sb[:, co, :].rearrange("p (s c) -> p s c", c=512)
            for s in range(8):
                nc.vector.bn_stats(out=st[:, s, :], in_=xvw[:, s, :])
            nc.vector.bn_aggr(out=mvall[:, co, :], in_=st[:])
        # group stats via tiny PE matmuls, batched over channel blocks
        nc.vector.tensor_copy(out=mv3[:, :, 0:2], in_=mvall[:])
        nc.vector.tensor_tensor(out=mv3[:, :, 2:3], in0=mvall[:, :, 0:1],
                                in1=mvall[:, :, 0:1], op=MUL)
        psg = bsh.tile([8, NCO, 3], F32, tag="b", name="psg")
        nc.tensor.matmul(psg.rearrange("g co s -> g (co s)"), gmat_sb,
                         mv3.rearrange("p co s -> p (co s)"),
                         start=True, stop=True)
        nc.vector.tensor_copy(out=gsm[:], in_=psg[:])
        nc.vector.tensor_scalar_mul(out=gmu84[:], in0=gsm[:, :, 0], scalar1=1.0 / 16.0)
        nc.vector.tensor_tensor(out=gvar84[:], in0=gsm[:, :, 1], in1=gsm[:, :, 2],
                                op=ADD)
        nc.vector.tensor_scalar_mul(out=gvar84[:], in0=gvar84[:], scalar1=1.0 / 16.0)
        nc.vector.tensor_tensor(out=srt84[:], in0=gmu84[:], in1=gmu84[:], op=MUL)
        nc.vector.tensor_tensor(out=gvar84[:], in0=gvar84[:], in1=srt84[:], op=SUB)
        # rstd = exp(-0.5*ln(var+eps)): stays on the Ln+Exp activation set
        nc.scalar.activation(out=srt84[:], in_=gvar84[:], func=AF.Ln, bias=eps8[:])
        nc.scalar.activation(out=grs[:, :, 0], in_=srt84[:], func=AF.Exp,
                             scale=-0.5)
        nc.vector.tensor_copy(out=grs[:, :, 1], in_=gmu84[:])
        psr = bsh.tile([128, NCO, 2], F32, tag="b", name="psr")
        nc.tensor.matmul(psr.rearrange("p co s -> p (co s)"), gmatT_sb[:],
                         grs.rearrange("g co s -> g (co s)"),
                         start=True, stop=True)
        nc.vector.tensor_copy(out=rg_pc[:], in_=psr[:])
        nc.vector.tensor_tensor(out=a_pc[:], in0=gnw_pc, in1=rg_pc[:, :, 0],
                                op=MUL)
        nc.vector.tensor_tensor(out=tmp_pc[:], in0=rg_pc[:, :, 1], in1=a_pc[:], op=MUL)
        nc.vector.tensor_tensor(out=d_pc[:], in0=gnb_pc, in1=tmp_pc[:], op=SUB)

        # qbias = wq_h @ d + bq ; wqs = wqT * a (columns scaled per channel)
        qb = bsh.tile([DH, 1], F32, tag="b", name="qb")
        for co in range(NCO):
            nc.tensor.matmul(qb[:], wqT_sb[:, co, :], d_pc[:, co:co + 1],
                             start=(co == 0), stop=(co == NCO - 1))
        nc.vector.tensor_tensor(out=qbias[:], in0=qb[:], in1=bq_sb, op=ADD)
        for co in range(NCO):
            nc.vector.tensor_scalar_mul(out=wqs_bf[:, co, :], in0=wqT_sb[:, co, :],
                                        scalar1=a_pc[:, co:co + 1])

        # ---------------- Q projection ----------------
        def q_proj(ic):
            sl = slice(ic * IC, (ic + 1) * IC)
            pq = bsh.tile([DH, IC], F32, tag="b", name="pq")
            for co in range(NCO):
                nc.tensor.matmul(pq[:], wqs_bf[:, co, :], xb_sb[:, co, sl],
                                 start=(co == 0), stop=(co == NCO - 1))
            with nc.allow_low_precision(reason="fp8 attention operand"):
                nc.vector.tensor_scalar(out=q8[:, sl], in0=pq[:],
                                        scalar1=qbias[:], scalar2=None, op0=ADD)

        # ---------------- attention blocks ----------------
        pav_tiles = {}
        av_pending = []
        deferred = []  # finish-epilogue pieces, pumped between groups

        def pump():
            if deferred:
                deferred.pop(0)()

        def emit_av(pav, g, ee):
            nc.tensor.matmul(pav[:], vT8[:, g, :, :], ee[:],
                             start=(g == 0), stop=(g == NG - 1),
                             perf_mode=DR)

        def att_block(ic, qt):
            """QK + exp for chunk ic over quarter qt's key tiles; the AV
            matmuls trail by AV_LAG groups (drained at qt==3)."""
            sl = slice(ic * IC, (ic + 1) * IC)
            if qt == 0:
                pav_tiles[ic] = psav.tile([96, IC], F32, tag="pav", name="pav")
            pav = pav_tiles[ic]
            for g in range(qt * (NG // 4), (qt + 1) * (NG // 4)):
                pump()
                jA, jB = 2 * g, 2 * g + 1
                tt = pst.tile([128, 2, IC], F32, tag="tps", name="tps")
                nc.tensor.matmul(tt[:, 0, :],
                                 bc2(k8[:, jA * 128:(jA + 1) * 128], DH, 128),
                                 bc2(q8[:, sl], DH, IC),
                                 start=True, stop=True, perf_mode=DR)
                nc.tensor.matmul(tt[:, 1, :],
                                 bc2(k8[:, jB * 128:(jB + 1) * 128], DH, 128),
                                 bc2(q8[:, sl], DH, IC),
                                 start=True, stop=True, perf_mode=DR)
                ee = expp.tile([128, 2, IC], FP8, tag="exp", name="exp")
                if g in TRICK_GROUPS:
                    bf = bfp.tile([128, 2, IC], F32, tag="bf", name="bf")
                    nc.vector.tensor_scalar(out=bf[:], in0=tt[:],
                                            scalar1=TRICK_A, scalar2=TRICK_B,
                                            op0=MUL, op1=ADD)
                    with nc.allow_low_precision(reason="bit-trick exp"):
                        nc.gpsimd.tensor_copy(out=ee[:].bitcast(U8), in_=bf[:])
                else:
                    nc.scalar.activation(out=ee[:], in_=tt[:], func=AF.Exp,
                                         scale=EXP_SCALE)
                av_pending.append((pav, g, ee))
                if len(av_pending) > AV_LAG:
                    emit_av(*av_pending.pop(0))
            if qt == 3:
                while av_pending:
                    emit_av(*av_pending.pop(0))

        def att_finish_head(ic):
            """normalize + attn8; the per-channel-block output projection
            pieces are queued on `deferred` and pumped between later groups
            so the DVE work spreads out instead of bursting."""
            sl = slice(ic * IC, (ic + 1) * IC)
            pav = pav_tiles.pop(ic)
            rden = attp.tile([1, IC], F32R, tag="rden", name="rden")
            with nc.allow_low_precision(reason="f32r matmul operand"):
                nc.vector.reciprocal(out=rden[:], in_=pav[64:65, :])
            rbp = bsh.tile([DH, IC], F32, tag="b", name="rbp")
            nc.tensor.matmul(rbp[:], ones_c0[:], rden[:], start=True, stop=True)
            rb = attp.tile([DH, IC], F32, tag="rb", name="rb")
            nc.vector.tensor_copy(out=rb[:], in_=rbp[:])
            attn8 = attp.tile([DH, IC], FP8, tag="attn", name="attn")
            with nc.allow_low_precision(reason="fp8 attention operand"):
                nc.vector.tensor_tensor(out=attn8[:], in0=pav[0:DH, :],
                                        in1=rb[:], op=MUL)

            def piece(ot, attn8=attn8, sl=sl):
                po = bsh.tile([128, IC], F32, tag="b", name="po")
                nc.tensor.matmul(po[:],
                                 bc2(woT8_sb[:, ot * 128:(ot + 1) * 128], DH, 128),
                                 bc2(attn8[:], DH, IC),
                                 start=True, stop=True, perf_mode=DR)
                pt = outp.tile([128, IC], BF16, tag="pt", name="pt")
                with nc.allow_low_precision(reason="bf16 partial output"):
                    nc.vector.scalar_tensor_tensor(out=pt[:], in0=po[:],
                                                   scalar=O_SCALE,
                                                   in1=xm_sb[:, ot, sl],
                                                   op0=MUL, op1=ADD)
                nc.sync.dma_start(out=pvw[:, ot, sl], in_=pt[:])

            for ot in range(NCO):
                deferred.append(lambda ot=ot: piece(ot))

        # ---------------- emission schedule ----------------
        q_proj(0)
        q_proj(1)
        for qt in range(4):
            att_block(0, qt)
        q_proj(2)
        for qt in range(4):
            att_block(1, qt)
        q_proj(3)
        nq = 4
        for ic in range(2, NIC):
            att_finish_head(ic - 2)
            if nq < NIC:
                q_proj(nq)
                nq += 1
            if ic < NIC - 1:
                for qt in range(4):
                    att_block(ic, qt)
            else:
                att_block(ic, 0)
                att_block(ic, 1)
                att_finish_head(ic - 1)  # chunk 6 finishes amid chunk 7
                att_block(ic, 2)
                att_block(ic, 3)
        while deferred:
            pump()
        att_finish_head(NIC - 1)
        while deferred:
            pump()

        for p in [stats_pool, outp, attp, bfp, expp, bsh, psav, pst, singles]:
            p.release()

    nc.compile()
    return nc


GMAT = (np.arange(128)[:, None] // 16 == np.arange(8)[None, :]).astype(np.float32)
GMATT = np.ascontiguousarray(GMAT.T)
NP8 = mybir.dt.np(FP8)
NPBF = mybir.dt.np(BF16)
VCOL = np.zeros((128, 32), dtype=np.float32)
VCOL[:, 0] = 1.0
VCOL = VCOL.astype(NP8)

_NC_CACHE = None


def get_nc():
    global _NC_CACHE
    if _NC_CACHE is None:
        _NC_CACHE = build_nc()
    return _NC_CACHE


def kernel(x, context, gn_w, gn_b, wq, bq, wk, bk, wv, bv, wo, bo):
    from concourse.bass_utils import run_bass_kernel_spmd

    x = np.asarray(x, dtype=np.float32)
    context = np.asarray(context, dtype=np.float32)
    gn_w = np.asarray(gn_w, dtype=np.float32)
    gn_b = np.asarray(gn_b, dtype=np.float32)
    wq = np.asarray(wq, dtype=np.float32)
    bq = np.asarray(bq, dtype=np.float32)
    wk = np.asarray(wk, dtype=np.float32)
    bk = np.asarray(bk, dtype=np.float32)
    wv = np.asarray(wv, dtype=np.float32)
    bv = np.asarray(bv, dtype=np.float32)
    wo = np.asarray(wo, dtype=np.float32)
    bo = np.asarray(bo, dtype=np.float32)

    B, C, H, W = x.shape
    xb2 = np.ascontiguousarray(x.reshape(C, H * W)).astype(NPBF)
    ctx82 = np.ascontiguousarray(context.reshape(CTXC, H * W)).astype(NP8)

    in_maps = []
    for h in range(NH):
        hs = slice(h * DH, (h + 1) * DH)
        pa = np.zeros((128, 21), dtype=np.float32)
        pa[:, 0:4] = gn_w.reshape(NCO, 128).T
        pa[:, 4:8] = gn_b.reshape(NCO, 128).T
        pa[:, 8:12] = bo.reshape(NCO, 128).T
        pa[:, 12:20] = GMAT
        pa[:, 20] = 1.0 if h == 0 else 0.0
        pb = np.stack([bq[hs], bk[hs] * W8, bv[hs]], axis=1).astype(np.float32)
        in_maps.append({
            "xb": xb2,
            "ctx8": ctx82,
            "wqT": np.ascontiguousarray(wq[hs, :].T),
            "wkT8": np.ascontiguousarray(wk[hs, :].T * W8).astype(NP8),
            "wvT8": np.ascontiguousarray(wv[hs, :].T * W8).astype(NP8),
            "woT8": np.ascontiguousarray(wo[:, hs].T * W8).astype(NP8),
            "woT": np.ascontiguousarray(wo[:, hs].T),
            "packA": pa,
            "packB": np.ascontiguousarray(pb),
            "gmatT": GMATT,
            "vcol": VCOL,
        })

    nc = get_nc()
    res = run_bass_kernel_spmd(nc, in_maps, core_ids=list(range(NH)))
    acc = np.zeros((C, H * W), dtype=np.float64)
    for h in range(NH):
        acc += res.results[h]["partial"].astype(np.float64)
    return acc.astype(np.float32).reshape(B, C, H, W)


# revision 36
# speedup vs baseline: 1.4360x; 1.0715x over previous
"""Trainium2 Bass kernel for a CrossAttentionBlock (GroupNorm + 8-head
cross-attention + output projection + residual).

Sharding: one attention head per NeuronCore (8 heads / 8 cores).  Each core
computes its head's partial output projection wo[:, h] @ attn_h; the host sums
the 8 partials (partial-sum unshard).  Residual and output bias are added on
core 0 only (mask input), so the host-side sum is a pure reduce.

v3: fp8e4 DoubleRow matmuls + multi-engine softmax exp.
 - x arrives bf16 (Q proj in bf16, GroupNorm stats, residual); context and
   attention weights arrive fp8e4, pre-scaled by 8 on the host so w*8 clears
   the e4m3 subnormal threshold.  Scale corrections fold into the exp scale
   and the output epilogue constant.
 - Scores tt = 2*k8^T q8 via stride-0 broadcast of the DoubleRow k-subtile
   dim; vT is computed directly as ctx8^T wv8 (keys on partitions) with a
   ones column riding the AV matmul as the softmax denominator (stationary
   width padded to 96 - DoubleRow needs a multiple of 32).
 - K/vT PSUM evacuation runs on ScalarE (activation Identity/Copy) so the
   whole K/V production overlaps the DVE GroupNorm-stats window; the
   residual+bias epilogue tiles (xm) are precomputed on GpSimd up front.
 - exp runs on ScalarE for 11/16 groups per chunk; the other 5 use a
   bit-trick path (DVE: b = tt*A + B in f32; GpSimd: f32->uint8 round, bits
   reinterpreted as e4m3 ~ exp) to spread the elementwise softmax cost over
   three engines.  The PWL error is a global constant times +-2.6% ripple;
   the constant cancels in softmax.

Self-contained: hardcodes all shapes from the problem spec.
"""

import math
import sys

sys.path.insert(0, "/opt/trn_rl_repo")

import numpy as np

import concourse.bass as bass
import concourse.tile as tile
from concourse import bacc, mybir

F32 = mybir.dt.float32
F32R = mybir.dt.float32r
BF16 = mybir.dt.bfloat16
FP8 = mybir.dt.float8e4
U8 = mybir.dt.uint8
DR = mybir.MatmulPerfMode.DoubleRow
AF = mybir.ActivationFunctionType

CH = 512          # x channels
CTXC = 768        # context channels
N = 4096          # spatial positions (64*64)
NH = 8            # heads
DH = 64           # head dim
G = 32            # groupnorm groups
EPS = 1e-5
NCO = CH // 128   # x channel blocks (4)
NCK = CTXC // 128  # ctx channel blocks (6)
IC = 512          # query-chunk size
NIC = N // IC     # 8 query chunks
NJT = N // 128    # 32 key tiles
NG = NJT // 2     # 16 exp groups (2 key tiles each) per chunk
SCALE = 1.0 / 8.0  # 1/sqrt(DH)
W8 = 8.0           # host-side fp8 weight pre-scale
# tt = 2 * (8 wk)^T q = 16 * (k.q); true score = SCALE * (k.q)
EXP_SCALE = SCALE / 16.0
# attn8 = C0 * pav[0:64] / den = C0 * 8 * sum(p v); attn8 ~ 1024 * sum(p v)
C0 = 128.0
# po = 2 * (8 wo)^T attn8 = 2*8*8*128 * o_true
O_SCALE = 1.0 / 16384.0
# bit-trick exp: uint8 bits of e4m3 ~ 8*(log2(v)+7); round mode on device
TRICK_A = EXP_SCALE * 8.0 / math.log(2.0)
TRICK_B = 55.8
# groups per chunk routed to the DVE+Pool bit-trick path
TRICK_GROUPS = frozenset({1, 4, 7, 10, 13})
# AV matmuls trail the QK/exp stream by this many groups so the in-order PE
# never stalls waiting for a fresh exp tile (the trick chain's DVE+Pool
# latency is ~3.5us, so the lag must cover ~5 groups of Act time)
AV_LAG = 5

ADD = mybir.AluOpType.add
SUB = mybir.AluOpType.subtract
MUL = mybir.AluOpType.mult


def bc2(ap, p, n):
    """Insert a stride-0 k-subtile dim of size 2 (DoubleRow dup trick)."""
    return ap.rearrange("p (o n) -> p o n", o=1).to_broadcast((p, 2, n))


def build_nc():
    nc = bacc.Bacc("TRN2", num_devices=8, debug=False)

    xb = nc.dram_tensor("xb", (CH, N), BF16, kind="ExternalInput")
    ctx8 = nc.dram_tensor("ctx8", (CTXC, N), FP8, kind="ExternalInput")
    wqT = nc.dram_tensor("wqT", (CH, DH), F32, kind="ExternalInput")
    wkT8 = nc.dram_tensor("wkT8", (CTXC, DH), FP8, kind="ExternalInput")
    wvT8 = nc.dram_tensor("wvT8", (CTXC, DH), FP8, kind="ExternalInput")
    woT8 = nc.dram_tensor("woT8", (DH, CH), FP8, kind="ExternalInput")
    # packA: gnw(0:4) gnb(4:8) bomv(8:12) gmat(12:20) msk(20:21), per-channel
    # row layout (bomv = bo*is0 + wo@bv, folded on host); packB: bq|bk8 cols
    packA = nc.dram_tensor("packA", (128, 21), F32, kind="ExternalInput")
    packB = nc.dram_tensor("packB", (DH, 2), F32, kind="ExternalInput")
    gmatT = nc.dram_tensor("gmatT", (8, 128), F32, kind="ExternalInput")
    vcol = nc.dram_tensor("vcol", (128, 32), FP8, kind="ExternalInput")
    partial = nc.dram_tensor("partial", (CH, N), BF16, kind="ExternalOutput")

    pvw = partial.rearrange("(co p) n -> p co n", p=128)

    with tile.TileContext(nc) as tc:
        singles = tc.alloc_tile_pool(name="singles", bufs=1)
        # PSUM: pst 3x2 + psav 1 + bsh 1 = 8 banks.  Three score slots let the
        # QK matmuls run two groups ahead of the exp stream, hiding the
        # slot-recycle semaphore latency behind two full exp instructions.
        pst = tc.alloc_tile_pool(name="pst", bufs=3, space="PSUM")
        psav = tc.alloc_tile_pool(name="psav", bufs=1, space="PSUM")
        bsh = tc.alloc_tile_pool(name="bsh", bufs=1, space="PSUM")
        expp = tc.alloc_tile_pool(name="expp", bufs=10)
        bfp = tc.alloc_tile_pool(name="bfp", bufs=4)
        attp = tc.alloc_tile_pool(name="attp", bufs=2)
        outp = tc.alloc_tile_pool(name="outp", bufs=2)
        stats_pool = tc.alloc_tile_pool(name="stats", bufs=2)

        # ---------------- persistent SBUF tiles ----------------
        xb_sb = singles.tile([128, NCO, N], BF16)
        ctx8_sb = singles.tile([128, NCK, N], FP8)
        xm_sb = singles.tile([128, NCO, N], F32)
        q8 = singles.tile([DH, N], FP8)
        k8 = singles.tile([DH, N], FP8)
        vT8 = singles.tile([128, NG, 2, 96], FP8)
        wqT_sb = singles.tile([128, NCO, DH], F32)
        wqs_bf = singles.tile([128, NCO, DH], BF16)
        wkT8_sb = singles.tile([128, NCK, DH], FP8)
        wvT8_sb = singles.tile([128, NCK, DH], FP8)
        woT8_sb = singles.tile([DH, CH], FP8)
        ones_c0 = singles.tile([1, DH], F32R)
        ones_f = singles.tile([1, DH], F32)
        packA_sb = singles.tile([128, 21], F32)
        packB_sb = singles.tile([DH, 2], F32)
        gmatT_sb = singles.tile([8, 128], F32)
        mvall = singles.tile([128, NCO, 2], F32)
        mv3 = singles.tile([128, NCO, 3], F32)
        gsm = singles.tile([8, NCO, 3], F32)
        gmu84 = singles.tile([8, NCO], F32)
        gvar84 = singles.tile([8, NCO], F32)
        srt84 = singles.tile([8, NCO], F32)
        grs = singles.tile([8, NCO, 2], F32)
        rg_pc = singles.tile([128, NCO, 2], F32)
        tmp_pc = singles.tile([128, NCO], F32)
        eps8 = singles.tile([8, 1], F32)
        a_pc = singles.tile([128, NCO], F32)
        d_pc = singles.tile([128, NCO], F32)
        qbias = singles.tile([DH, 1], F32)
        expwarm = singles.tile([1, 1], F32)
        # views into the packed small-constants tiles
        gnw_pc = packA_sb[:, 0:4]
        gnb_pc = packA_sb[:, 4:8]
        bomv = packA_sb[:, 8:12]
        gmat_sb = packA_sb[:, 12:20]
        msk = packA_sb[:, 20:21]
        bq_sb = packB_sb[:, 0:1]
        kb_sb = packB_sb[:, 1:2]

        # ---------------- input loads ----------------
        # DMA issue on SP is sequential (~650ns each): small packs first so
        # the bias chain unblocks, then x blocks (GroupNorm stats path), the
        # K/V weights + context (K/vT production), then the rest.
        xv = xb.rearrange("(co p) n -> p co n", p=128)
        cv = ctx8.rearrange("(ck p) n -> p ck n", p=128)
        nc.sync.dma_start(out=packA_sb[:], in_=packA[:])
        nc.sync.dma_start(out=packB_sb[:], in_=packB[:])
        for co in range(NCO):
            nc.sync.dma_start(out=xb_sb[:, co, :], in_=xv[:, co, :])
        nc.sync.dma_start(out=wkT8_sb[:], in_=wkT8.rearrange("(ck p) d -> p ck d", p=128))
        nc.sync.dma_start(out=wvT8_sb[:], in_=wvT8.rearrange("(ck p) d -> p ck d", p=128))
        for ck in range(NCK):
            nc.sync.dma_start(out=ctx8_sb[:, ck, :], in_=cv[:, ck, :])
        nc.sync.dma_start(out=wqT_sb[:], in_=wqT.rearrange("(co p) d -> p co d", p=128))
        nc.sync.dma_start(out=woT8_sb[:], in_=woT8[:])
        nc.sync.dma_start(out=gmatT_sb[:], in_=gmatT[:])
        # vT pad columns 64:96 = [1, 0, ..., 0] via broadcast DMA
        nc.sync.dma_start(
            out=vT8[:, :, :, 64:96].rearrange("p a b n -> p (a b) n"),
            in_=vcol.rearrange("p (a n) -> p a n", a=1).to_broadcast(
                (128, NG * 2, 32)))

        # small constants + activation table warm (Ln+Exp share a set)
        nc.vector.memset(ones_f[:], C0)
        nc.vector.tensor_copy(out=ones_c0[:], in_=ones_f[:])
        nc.vector.memset(eps8[:], EPS)
        nc.vector.memset(expwarm[:], 1.0)
        nc.scalar.activation(out=expwarm[:], in_=expwarm[:], func=AF.Ln)
        nc.scalar.activation(out=expwarm[:], in_=expwarm[:], func=AF.Exp)

        # ---------------- K and vT production (PE + ScalarE) ----------------
        def kv_quarter(qt):
            for lc in range(2):
                ic = qt * 2 + lc
                sl = slice(ic * IC, (ic + 1) * IC)
                pk = pst.tile([DH, IC], F32, tag="tps", name="pk")
                for j in range(NCK // 2):
                    nc.tensor.matmul(pk[:], wkT8_sb[:, 2 * j:2 * j + 2, :],
                                     ctx8_sb[:, 2 * j:2 * j + 2, sl],
                                     start=(j == 0), stop=(j == NCK // 2 - 1),
                                     perf_mode=DR)
                with nc.allow_low_precision(reason="fp8 attention operand"):
                    nc.scalar.activation(out=k8[:, sl], in_=pk[:],
                                         func=AF.Identity, bias=kb_sb)
            for h in range(2):  # two 4-key-tile batches per quarter
                kt0 = qt * 8 + h * 4
                pvt = pst.tile([128, 4, DH], F32, tag="tps", name="pvt")
                for i in range(4):
                    ks = slice((kt0 + i) * 128, (kt0 + i + 1) * 128)
                    for j in range(NCK // 2):
                        nc.tensor.matmul(pvt[:, i, :],
                                         ctx8_sb[:, 2 * j:2 * j + 2, ks],
                                         wvT8_sb[:, 2 * j:2 * j + 2, :],
                                         start=(j == 0),
                                         stop=(j == NCK // 2 - 1),
                                         perf_mode=DR)
                dst = vT8[:, kt0 // 2:kt0 // 2 + 2, :, 0:DH].rearrange(
                    "p a b d -> p (a b) d")
                with nc.allow_low_precision(reason="fp8 attention operand"):
                    nc.scalar.activation(out=dst, in_=pvt[:], func=AF.Copy)

        for qt in range(4):
            kv_quarter(qt)

        # residual epilogue tiles on GpSimd: co0/co1 halves up front, the
        # co2/co3 halves spread into chunk-0's block emission so they do not
        # delay the chunk-0/1 bit-trick converts on the Pool queue
        def xm_half(co, hh):
            hsl = slice(hh * (N // 2), (hh + 1) * (N // 2))
            nc.gpsimd.tensor_scalar(out=xm_sb[:, co, hsl],
                                    in0=xb_sb[:, co, hsl],
                                    scalar1=msk, scalar2=bomv[:, co:co + 1],
                                    op0=MUL, op1=ADD)

        for co in range(2):
            for hh in range(2):
                xm_half(co, hh)
        pool_prelude = [lambda co=co, hh=hh: xm_half(co, hh)
                        for co in (2, 3) for hh in range(2)]

        # ---------------- groupnorm statistics (DVE) ----------------
        for co in range(NCO):
            st = stats_pool.tile([128, 8, 6], F32)
            xvw = xb_sb[:, co, :].rearrange("p (s c) -> p s c", c=512)
            for s in range(8):
                nc.vector.bn_stats(out=st[:, s, :], in_=xvw[:, s, :])
            nc.vector.bn_aggr(out=mvall[:, co, :], in_=st[:])
        # group stats via tiny PE matmuls, batched over channel blocks
        nc.vector.tensor_copy(out=mv3[:, :, 0:2], in_=mvall[:])
        nc.vector.tensor_tensor(out=mv3[:, :, 2:3], in0=mvall[:, :, 0:1],
                                in1=mvall[:, :, 0:1], op=MUL)
        psg = bsh.tile([8, NCO, 3], F32, tag="b", name="psg")
        nc.tensor.matmul(psg.rearrange("g co s -> g (co s)"), gmat_sb,
                         mv3.rearrange("p co s -> p (co s)"),
                         start=True, stop=True)
        nc.vector.tensor_copy(out=gsm[:], in_=psg[:])
        nc.vector.tensor_scalar_mul(out=gmu84[:], in0=gsm[:, :, 0], scalar1=1.0 / 16.0)
        nc.vector.tensor_tensor(out=gvar84[:], in0=gsm[:, :, 1], in1=gsm[:, :, 2],
                                op=ADD)
        nc.vector.tensor_scalar_mul(out=gvar84[:], in0=gvar84[:], scalar1=1.0 / 16.0)
        nc.vector.tensor_tensor(out=srt84[:], in0=gmu84[:], in1=gmu84[:], op=MUL)
        nc.vector.tensor_tensor(out=gvar84[:], in0=gvar84[:], in1=srt84[:], op=SUB)
        # rstd = exp(-0.5*ln(var+eps)): stays on the Ln+Exp activation set
        nc.scalar.activation(out=srt84[:], in_=gvar84[:], func=AF.Ln, bias=eps8[:])
        nc.scalar.activation(out=grs[:, :, 0], in_=srt84[:], func=AF.Exp,
                             scale=-0.5)
        nc.vector.tensor_copy(out=grs[:, :, 1], in_=gmu84[:])
        psr = bsh.tile([128, NCO, 2], F32, tag="b", name="psr")
        nc.tensor.matmul(psr.rearrange("p co s -> p (co s)"), gmatT_sb[:],
                         grs.rearrange("g co s -> g (co s)"),
                         start=True, stop=True)
        nc.vector.tensor_copy(out=rg_pc[:], in_=psr[:])
        nc.vector.tensor_tensor(out=a_pc[:], in0=gnw_pc, in1=rg_pc[:, :, 0],
                                op=MUL)
        nc.vector.tensor_tensor(out=tmp_pc[:], in0=rg_pc[:, :, 1], in1=a_pc[:], op=MUL)
        nc.vector.tensor_tensor(out=d_pc[:], in0=gnb_pc, in1=tmp_pc[:], op=SUB)

        # qbias = wq_h @ d + bq ; wqs = wqT * a (columns scaled per channel)
        qb = bsh.tile([DH, 1], F32, tag="b", name="qb")
        for co in range(NCO):
            nc.tensor.matmul(qb[:], wqT_sb[:, co, :], d_pc[:, co:co + 1],
                             start=(co == 0), stop=(co == NCO - 1))
        nc.vector.tensor_tensor(out=qbias[:], in0=qb[:], in1=bq_sb, op=ADD)
        for co in range(NCO):
            nc.vector.tensor_scalar_mul(out=wqs_bf[:, co, :], in0=wqT_sb[:, co, :],
                                        scalar1=a_pc[:, co:co + 1])

        # ---------------- Q projection ----------------
        def q_proj(ic):
            sl = slice(ic * IC, (ic + 1) * IC)
            pq = bsh.tile([DH, IC], F32, tag="b", name="pq")
            for co in range(NCO):
                nc.tensor.matmul(pq[:], wqs_bf[:, co, :], xb_sb[:, co, sl],
                                 start=(co == 0), stop=(co == NCO - 1))
            with nc.allow_low_precision(reason="fp8 attention operand"):
                nc.vector.tensor_scalar(out=q8[:, sl], in0=pq[:],
                                        scalar1=qbias[:], scalar2=None, op0=ADD)

        # ---------------- attention blocks ----------------
        pav_tiles = {}
        av_pending = []
        deferred = []  # finish-epilogue pieces, pumped between groups

        def pump():
            if deferred:
                deferred.pop(0)()

        def emit_av(pav, g, ee):
            nc.tensor.matmul(pav[:], vT8[:, g, :, :], ee[:],
                             start=(g == 0), stop=(g == NG - 1),
                             perf_mode=DR)

        def att_block(ic, qt):
            """QK + exp for chunk ic over quarter qt's key tiles; the AV
            matmuls trail by AV_LAG groups (drained at qt==3)."""
            sl = slice(ic * IC, (ic + 1) * IC)
            if qt == 0:
                pav_tiles[ic] = psav.tile([96, IC], F32, tag="pav", name="pav")
            pav = pav_tiles[ic]
            if pool_prelude:
                pool_prelude.pop(0)()
            for g in range(qt * (NG // 4), (qt + 1) * (NG // 4)):
                pump()
                jA, jB = 2 * g, 2 * g + 1
                tt = pst.tile([128, 2, IC], F32, tag="tps", name="tps")
                nc.tensor.matmul(tt[:, 0, :],
                                 bc2(k8[:, jA * 128:(jA + 1) * 128], DH, 128),
                                 bc2(q8[:, sl], DH, IC),
                                 start=True, stop=True, perf_mode=DR)
                nc.tensor.matmul(tt[:, 1, :],
                                 bc2(k8[:, jB * 128:(jB + 1) * 128], DH, 128),
                                 bc2(q8[:, sl], DH, IC),
                                 start=True, stop=True, perf_mode=DR)
                ee = expp.tile([128, 2, IC], FP8, tag="exp", name="exp")
                if g in TRICK_GROUPS:
                    bf = bfp.tile([128, 2, IC], F32, tag="bf", name="bf")
                    nc.vector.tensor_scalar(out=bf[:], in0=tt[:],
                                            scalar1=TRICK_A, scalar2=TRICK_B,
                                            op0=MUL, op1=ADD)
                    with nc.allow_low_precision(reason="bit-trick exp"):
                        nc.gpsimd.tensor_copy(out=ee[:].bitcast(U8), in_=bf[:])
                else:
                    nc.scalar.activation(out=ee[:], in_=tt[:], func=AF.Exp,
                                         scale=EXP_SCALE)
                av_pending.append((pav, g, ee))
                if len(av_pending) > AV_LAG:
                    emit_av(*av_pending.pop(0))
            if qt == 3:
                while av_pending:
                    emit_av(*av_pending.pop(0))

        def att_finish_head(ic):
            """normalize + attn8; the per-channel-block output projection
            pieces are queued on `deferred` and pumped between later groups
            so the DVE work spreads out instead of bursting."""
            sl = slice(ic * IC, (ic + 1) * IC)
            pav = pav_tiles.pop(ic)
            rden = attp.tile([1, IC], F32R, tag="rden", name="rden")
            with nc.allow_low_precision(reason="f32r matmul operand"):
                nc.vector.reciprocal(out=rden[:], in_=pav[64:65, :])
            rbp = bsh.tile([DH, IC], F32, tag="b", name="rbp")
            nc.tensor.matmul(rbp[:], ones_c0[:], rden[:], start=True, stop=True)
            rb = attp.tile([DH, IC], F32, tag="rb", name="rb")
            nc.vector.tensor_copy(out=rb[:], in_=rbp[:])
            attn8 = attp.tile([DH, IC], FP8, tag="attn", name="attn")
            with nc.allow_low_precision(reason="fp8 attention operand"):
                nc.vector.tensor_tensor(out=attn8[:], in0=pav[0:DH, :],
                                        in1=rb[:], op=MUL)

            def piece(ot, attn8=attn8, sl=sl):
                po = bsh.tile([128, IC], F32, tag="b", name="po")
                nc.tensor.matmul(po[:],
                                 bc2(woT8_sb[:, ot * 128:(ot + 1) * 128], DH, 128),
                                 bc2(attn8[:], DH, IC),
                                 start=True, stop=True, perf_mode=DR)
                pt = outp.tile([128, IC], BF16, tag="pt", name="pt")
                with nc.allow_low_precision(reason="bf16 partial output"):
                    nc.vector.scalar_tensor_tensor(out=pt[:], in0=po[:],
                                                   scalar=O_SCALE,
                                                   in1=xm_sb[:, ot, sl],
                                                   op0=MUL, op1=ADD)
                nc.sync.dma_start(out=pvw[:, ot, sl], in_=pt[:])

            for ot in range(NCO):
                deferred.append(lambda ot=ot: piece(ot))

        # ---------------- emission schedule ----------------
        q_proj(0)
        q_proj(1)
        for qt in range(4):
            att_block(0, qt)
        q_proj(2)
        for qt in range(4):
            att_block(1, qt)
        q_proj(3)
        nq = 4
        for ic in range(2, NIC):
            att_finish_head(ic - 2)
            if nq < NIC:
                q_proj(nq)
                nq += 1
            if ic < NIC - 1:
                for qt in range(4):
                    att_block(ic, qt)
            else:
                att_block(ic, 0)
                att_block(ic, 1)
                att_finish_head(ic - 1)  # chunk 6 finishes amid chunk 7
                att_block(ic, 2)
                att_block(ic, 3)
        while deferred:
            pump()
        att_finish_head(NIC - 1)
        while deferred:
            pump()

        for p in [stats_pool, outp, attp, bfp, expp, bsh, psav, pst, singles]:
            p.release()

    nc.compile()
    return nc


GMAT = (np.arange(128)[:, None] // 16 == np.arange(8)[None, :]).astype(np.float32)
GMATT = np.ascontiguousarray(GMAT.T)
NP8 = mybir.dt.np(FP8)
NPBF = mybir.dt.np(BF16)
VCOL = np.zeros((128, 32), dtype=np.float32)
VCOL[:, 0] = 1.0
VCOL = VCOL.astype(NP8)

_NC_CACHE = None


def get_nc():
    global _NC_CACHE
    if _NC_CACHE is None:
        _NC_CACHE = build_nc()
    return _NC_CACHE


def kernel(x, context, gn_w, gn_b, wq, bq, wk, bk, wv, bv, wo, bo):
    from concourse.bass_utils import run_bass_kernel_spmd

    x = np.asarray(x, dtype=np.float32)
    context = np.asarray(context, dtype=np.float32)
    gn_w = np.asarray(gn_w, dtype=np.float32)
    gn_b = np.asarray(gn_b, dtype=np.float32)
    wq = np.asarray(wq, dtype=np.float32)
    bq = np.asarray(bq, dtype=np.float32)
    wk = np.asarray(wk, dtype=np.float32)
    bk = np.asarray(bk, dtype=np.float32)
    wv = np.asarray(wv, dtype=np.float32)
    bv = np.asarray(bv, dtype=np.float32)
    wo = np.asarray(wo, dtype=np.float32)
    bo = np.asarray(bo, dtype=np.float32)

    B, C, H, W = x.shape
    xb2 = np.ascontiguousarray(x.reshape(C, H * W)).astype(NPBF)
    ctx82 = np.ascontiguousarray(context.reshape(CTXC, H * W)).astype(NP8)

    in_maps = []
    for h in range(NH):
        hs = slice(h * DH, (h + 1) * DH)
        pa = np.zeros((128, 21), dtype=np.float32)
        pa[:, 0:4] = gn_w.reshape(NCO, 128).T
        pa[:, 4:8] = gn_b.reshape(NCO, 128).T
        bomv_h = bo * (1.0 if h == 0 else 0.0) + wo[:, hs] @ bv[hs]
        pa[:, 8:12] = bomv_h.reshape(NCO, 128).T
        pa[:, 12:20] = GMAT
        pa[:, 20] = 1.0 if h == 0 else 0.0
        pb = np.stack([bq[hs], bk[hs] * W8], axis=1).astype(np.float32)
        in_maps.append({
            "xb": xb2,
            "ctx8": ctx82,
            "wqT": np.ascontiguousarray(wq[hs, :].T),
            "wkT8": np.ascontiguousarray(wk[hs, :].T * W8).astype(NP8),
            "wvT8": np.ascontiguousarray(wv[hs, :].T * W8).astype(NP8),
            "woT8": np.ascontiguousarray(wo[:, hs].T * W8).astype(NP8),
            "packA": pa,
            "packB": np.ascontiguousarray(pb),
            "gmatT": GMATT,
            "vcol": VCOL,
        })

    nc = get_nc()
    res = run_bass_kernel_spmd(nc, in_maps, core_ids=list(range(NH)))
    acc = np.zeros((C, H * W), dtype=np.float64)
    for h in range(NH):
        acc += res.results[h]["partial"].astype(np.float64)
    return acc.astype(np.float32).reshape(B, C, H, W)


# revision 39
# speedup vs baseline: 1.5206x; 1.0589x over previous
"""Trainium2 Bass kernel for a CrossAttentionBlock (GroupNorm + 8-head
cross-attention + output projection + residual).

Sharding: one attention head per NeuronCore (8 heads / 8 cores).  Each core
computes its head's partial output projection wo[:, h] @ attn_h; the host sums
the 8 partials (partial-sum unshard).  Residual and output bias are added on
core 0 only (mask input), so the host-side sum is a pure reduce.

v3: fp8e4 DoubleRow matmuls + multi-engine softmax exp.
 - x arrives bf16 (Q proj in bf16, GroupNorm stats, residual); context and
   attention weights arrive fp8e4, pre-scaled by 8 on the host so w*8 clears
   the e4m3 subnormal threshold.  Scale corrections fold into the exp scale
   and the output epilogue constant.
 - Scores tt = 2*k8^T q8 via stride-0 broadcast of the DoubleRow k-subtile
   dim; vT is computed directly as ctx8^T wv8 (keys on partitions) with a
   ones column riding the AV matmul as the softmax denominator (stationary
   width padded to 96 - DoubleRow needs a multiple of 32).
 - K/vT PSUM evacuation runs on ScalarE (activation Identity/Copy) so the
   whole K/V production overlaps the DVE GroupNorm-stats window; the
   residual+bias epilogue tiles (xm) are precomputed on GpSimd up front.
 - exp runs on ScalarE for 11/16 groups per chunk; the other 5 use a
   bit-trick path (DVE: b = tt*A + B in f32; GpSimd: f32->uint8 round, bits
   reinterpreted as e4m3 ~ exp) to spread the elementwise softmax cost over
   three engines.  The PWL error is a global constant times +-2.6% ripple;
   the constant cancels in softmax.

Self-contained: hardcodes all shapes from the problem spec.
"""

import math
import sys

sys.path.insert(0, "/opt/trn_rl_repo")

import numpy as np

import concourse.bass as bass
import concourse.tile as tile
from concourse import bacc, mybir

F32 = mybir.dt.float32
F32R = mybir.dt.float32r
BF16 = mybir.dt.bfloat16
FP8 = mybir.dt.float8e4
U8 = mybir.dt.uint8
DR = mybir.MatmulPerfMode.DoubleRow
AF = mybir.ActivationFunctionType

CH = 512          # x channels
CTXC = 768        # context channels
N = 4096          # spatial positions (64*64)
NH = 8            # heads
DH = 64           # head dim
G = 32            # groupnorm groups
EPS = 1e-5
NCO = CH // 128   # x channel blocks (4)
NCK = CTXC // 128  # ctx channel blocks (6)
IC = 512          # query-chunk size
NIC = N // IC     # 8 query chunks
NJT = N // 128    # 32 key tiles
NG = NJT // 2     # 16 exp groups (2 key tiles each) per chunk
SCALE = 1.0 / 8.0  # 1/sqrt(DH)
W8 = 8.0           # host-side fp8 weight pre-scale
# tt = 2 * (8 wk)^T q = 16 * (k.q); true score = SCALE * (k.q)
EXP_SCALE = SCALE / 16.0
# attn8 = C0 * pav[0:64] / den = C0 * 8 * sum(p v); attn8 ~ 1024 * sum(p v)
C0 = 128.0
# po = 2 * (8 wo)^T attn8 = 2*8*8*128 * o_true
O_SCALE = 1.0 / 16384.0
# bit-trick exp: uint8 bits of e4m3 ~ 8*(log2(v)+7); round mode on device
TRICK_A = EXP_SCALE * 8.0 / math.log(2.0)
TRICK_B = 55.8
# groups per chunk routed to the DVE+Pool bit-trick path
TRICK_GROUPS = frozenset({1, 4, 7, 10, 13})
# AV matmuls trail the QK/exp stream by this many groups so the in-order PE
# never stalls waiting for a fresh exp tile (the trick chain's DVE+Pool
# latency is ~3.5us, so the lag must cover ~5 groups of Act time)
AV_LAG = 5

ADD = mybir.AluOpType.add
SUB = mybir.AluOpType.subtract
MUL = mybir.AluOpType.mult


def bc2(ap, p, n):
    """Insert a stride-0 k-subtile dim of size 2 (DoubleRow dup trick)."""
    return ap.rearrange("p (o n) -> p o n", o=1).to_broadcast((p, 2, n))


def build_nc():
    nc = bacc.Bacc("TRN2", num_devices=8, debug=False)

    xb = nc.dram_tensor("xb", (CH, N), BF16, kind="ExternalInput")
    ctx8 = nc.dram_tensor("ctx8", (CTXC, N), FP8, kind="ExternalInput")
    wqT = nc.dram_tensor("wqT", (CH, DH), F32, kind="ExternalInput")
    wkT8 = nc.dram_tensor("wkT8", (CTXC, DH), FP8, kind="ExternalInput")
    wvT8 = nc.dram_tensor("wvT8", (CTXC, DH), FP8, kind="ExternalInput")
    woT8 = nc.dram_tensor("woT8", (DH, CH), FP8, kind="ExternalInput")
    # packA: gnw(0:4) gnb(4:8) bomv(8:12) gmat(12:20) msk(20:21), per-channel
    # row layout (bomv = bo*is0 + wo@bv, folded on host); packB: bq|bk8 cols
    packA = nc.dram_tensor("packA", (128, 21), F32, kind="ExternalInput")
    packB = nc.dram_tensor("packB", (DH, 2), F32, kind="ExternalInput")
    gmatT = nc.dram_tensor("gmatT", (8, 128), F32, kind="ExternalInput")
    vcol = nc.dram_tensor("vcol", (128, 32), FP8, kind="ExternalInput")
    partial = nc.dram_tensor("partial", (CH, N), BF16, kind="ExternalOutput")

    pvw = partial.rearrange("(co p) n -> p co n", p=128)

    with tile.TileContext(nc) as tc:
        singles = tc.alloc_tile_pool(name="singles", bufs=1)
        # PSUM: pst 3x2 + psav 1 + bsh 1 = 8 banks.  Three score slots let the
        # QK matmuls run two groups ahead of the exp stream, hiding the
        # slot-recycle semaphore latency behind two full exp instructions.
        pst = tc.alloc_tile_pool(name="pst", bufs=3, space="PSUM")
        psav = tc.alloc_tile_pool(name="psav", bufs=1, space="PSUM")
        bsh = tc.alloc_tile_pool(name="bsh", bufs=1, space="PSUM")
        expp = tc.alloc_tile_pool(name="expp", bufs=10)
        bfp = tc.alloc_tile_pool(name="bfp", bufs=4)
        attp = tc.alloc_tile_pool(name="attp", bufs=2)
        outp = tc.alloc_tile_pool(name="outp", bufs=2)
        stats_pool = tc.alloc_tile_pool(name="stats", bufs=2)

        # ---------------- persistent SBUF tiles ----------------
        xb_sb = singles.tile([128, NCO, N], BF16)
        ctx8_sb = singles.tile([128, NCK, N], FP8)
        xm_sb = singles.tile([128, NCO, N], F32)
        q8 = singles.tile([DH, N], FP8)
        k8 = singles.tile([DH, N], FP8)
        vT8 = singles.tile([128, NG, 2, 96], FP8)
        wqT_sb = singles.tile([128, NCO, DH], F32)
        wqs_bf = singles.tile([128, NCO, DH], BF16)
        wkT8_sb = singles.tile([128, NCK, DH], FP8)
        wvT8_sb = singles.tile([128, NCK, DH], FP8)
        woT8_sb = singles.tile([DH, CH], FP8)
        ones_c0 = singles.tile([1, DH], F32R)
        ones_f = singles.tile([1, DH], F32)
        packA_sb = singles.tile([128, 21], F32)
        packB_sb = singles.tile([DH, 2], F32)
        gmatT_sb = singles.tile([8, 128], F32)
        mvall = singles.tile([128, NCO, 2], F32)
        mv3 = singles.tile([128, NCO, 3], F32)
        gsm = singles.tile([8, NCO, 3], F32)
        gmu84 = singles.tile([8, NCO], F32)
        gvar84 = singles.tile([8, NCO], F32)
        srt84 = singles.tile([8, NCO], F32)
        grs = singles.tile([8, NCO, 2], F32)
        rg_pc = singles.tile([128, NCO, 2], F32)
        tmp_pc = singles.tile([128, NCO], F32)
        eps8 = singles.tile([8, 1], F32)
        a_pc = singles.tile([128, NCO], F32)
        d_pc = singles.tile([128, NCO], F32)
        qbias = singles.tile([DH, 1], F32)
        expwarm = singles.tile([1, 1], F32)
        scr_sb = singles.tile([128, N], F32)
        ssum2 = singles.tile([128, 2], F32)
        # views into the packed small-constants tiles
        gnw_pc = packA_sb[:, 0:4]
        gnb_pc = packA_sb[:, 4:8]
        bomv = packA_sb[:, 8:12]
        gmat_sb = packA_sb[:, 12:20]
        msk = packA_sb[:, 20:21]
        bq_sb = packB_sb[:, 0:1]
        kb_sb = packB_sb[:, 1:2]

        # ---------------- input loads ----------------
        # DMA issue on SP is sequential (~650ns each): small packs first so
        # the bias chain unblocks, then x blocks (GroupNorm stats path), the
        # K/V weights + context (K/vT production), then the rest.
        xv = xb.rearrange("(co p) n -> p co n", p=128)
        cv = ctx8.rearrange("(ck p) n -> p ck n", p=128)
        def dma_ctx_quarter(qt):
            cs = slice(qt * (N // 4), (qt + 1) * (N // 4))
            for ck in range(NCK):
                nc.sync.dma_start(out=ctx8_sb[:, ck, cs], in_=cv[:, ck, cs])

        nc.sync.dma_start(out=packA_sb[:], in_=packA[:])
        nc.sync.dma_start(out=packB_sb[:], in_=packB[:])
        nc.sync.dma_start(out=xb_sb[:, 0, :], in_=xv[:, 0, :])
        nc.sync.dma_start(out=xb_sb[:, 1, :], in_=xv[:, 1, :])
        nc.sync.dma_start(out=wkT8_sb[:], in_=wkT8.rearrange("(ck p) d -> p ck d", p=128))
        nc.sync.dma_start(out=wvT8_sb[:], in_=wvT8.rearrange("(ck p) d -> p ck d", p=128))
        dma_ctx_quarter(0)
        nc.sync.dma_start(out=xb_sb[:, 2, :], in_=xv[:, 2, :])
        nc.sync.dma_start(out=xb_sb[:, 3, :], in_=xv[:, 3, :])
        nc.sync.dma_start(out=wqT_sb[:], in_=wqT.rearrange("(co p) d -> p co d", p=128))
        nc.sync.dma_start(out=gmatT_sb[:], in_=gmatT[:])
        dma_ctx_quarter(1)
        nc.sync.dma_start(out=woT8_sb[:], in_=woT8[:])
        dma_ctx_quarter(2)
        dma_ctx_quarter(3)
        # vT pad columns 64:96 = [1, 0, ..., 0] via broadcast DMA
        nc.sync.dma_start(
            out=vT8[:, :, :, 64:96].rearrange("p a b n -> p (a b) n"),
            in_=vcol.rearrange("p (a n) -> p a n", a=1).to_broadcast(
                (128, NG * 2, 32)))

        # small constants + activation table warm (Ln+Exp share a set)
        nc.vector.memset(ones_f[:], C0)
        nc.vector.tensor_copy(out=ones_c0[:], in_=ones_f[:])
        nc.vector.memset(eps8[:], EPS)
        nc.vector.memset(expwarm[:], 1.0)
        nc.scalar.activation(out=expwarm[:], in_=expwarm[:], func=AF.Ln)
        nc.scalar.activation(out=expwarm[:], in_=expwarm[:], func=AF.Exp)

        # ---------------- K and vT production matmuls (PE) ----------------
        # Evacuation to SBUF runs on DVE after the stats blocks; PE may stall
        # on pst slots meanwhile, which is harmless (it has nothing else to
        # do before the Q projection).
        def kv_mm_quarter(qt):
            tiles = []
            for lc in range(2):
                ic = qt * 2 + lc
                sl = slice(ic * IC, (ic + 1) * IC)
                pk = pst.tile([DH, IC], F32, tag="tps", name="pk")
                for j in range(NCK // 2):
                    nc.tensor.matmul(pk[:], wkT8_sb[:, 2 * j:2 * j + 2, :],
                                     ctx8_sb[:, 2 * j:2 * j + 2, sl],
                                     start=(j == 0), stop=(j == NCK // 2 - 1),
                                     perf_mode=DR)
                tiles.append(("k", pk, sl))
            for h in range(2):  # two 4-key-tile batches per quarter
                kt0 = qt * 8 + h * 4
                pvt = pst.tile([128, 4, DH], F32, tag="tps", name="pvt")
                for i in range(4):
                    ks = slice((kt0 + i) * 128, (kt0 + i + 1) * 128)
                    for j in range(NCK // 2):
                        nc.tensor.matmul(pvt[:, i, :],
                                         ctx8_sb[:, 2 * j:2 * j + 2, ks],
                                         wvT8_sb[:, 2 * j:2 * j + 2, :],
                                         start=(j == 0),
                                         stop=(j == NCK // 2 - 1),
                                         perf_mode=DR)
                tiles.append(("v", pvt, kt0))
            return tiles

        def kv_evac(tiles):
            for kind, tl, aux in tiles:
                if kind == "k":
                    with nc.allow_low_precision(reason="fp8 attention operand"):
                        nc.vector.tensor_scalar(out=k8[:, aux], in0=tl[:],
                                                scalar1=kb_sb, scalar2=None,
                                                op0=ADD)
                else:
                    dst = vT8[:, aux // 2:aux // 2 + 2, :, 0:DH].rearrange(
                        "p a b d -> p (a b) d")
                    nc.vector.tensor_copy(out=dst, in_=tl[:])

        kv_tiles = [kv_mm_quarter(qt) for qt in range(4)]

        # residual epilogue tiles on GpSimd: co0/co1 halves up front, the
        # co2/co3 halves spread into chunk-0's block emission so they do not
        # delay the chunk-0/1 bit-trick converts on the Pool queue
        def xm_half(co, hh):
            hsl = slice(hh * (N // 2), (hh + 1) * (N // 2))
            nc.gpsimd.tensor_scalar(out=xm_sb[:, co, hsl],
                                    in0=xb_sb[:, co, hsl],
                                    scalar1=msk, scalar2=bomv[:, co:co + 1],
                                    op0=MUL, op1=ADD)

        for co in range(2):
            for hh in range(2):
                xm_half(co, hh)
        pool_prelude = [lambda co=co, hh=hh: xm_half(co, hh)
                        for co in (2, 3) for hh in range(2)]

        # ---------------- groupnorm statistics ----------------
        # DVE: bn_stats for channel blocks 0, 1, 3.  ScalarE (idle during
        # startup): two accumulate passes (sum, sum of squares) for block 2,
        # shortening the serial DVE stats chain.
        for co in (0, 1, 3):
            st = stats_pool.tile([128, 8, 6], F32)
            xvw = xb_sb[:, co, :].rearrange("p (s c) -> p s c", c=512)
            for s in range(8):
                nc.vector.bn_stats(out=st[:, s, :], in_=xvw[:, s, :])
            nc.vector.bn_aggr(out=mvall[:, co, :], in_=st[:])
        nc.scalar.activation(out=scr_sb[:], in_=xb_sb[:, 2, :], func=AF.Copy,
                             accum_out=ssum2[:, 0:1])
        nc.scalar.activation(out=scr_sb[:], in_=xb_sb[:, 2, :], func=AF.Square,
                             accum_out=ssum2[:, 1:2])

        kv_evac(kv_tiles[0])

        # mv3 rows: (mean, var, mean^2) per channel
        nc.vector.tensor_copy(out=mv3[:, 0:2, 0:2], in_=mvall[:, 0:2, :])
        nc.vector.tensor_copy(out=mv3[:, 3:4, 0:2], in_=mvall[:, 3:4, :])
        nc.vector.tensor_tensor(out=mv3[:, 0:2, 2:3], in0=mvall[:, 0:2, 0:1],
                                in1=mvall[:, 0:2, 0:1], op=MUL)
        nc.vector.tensor_tensor(out=mv3[:, 3:4, 2:3], in0=mvall[:, 3:4, 0:1],
                                in1=mvall[:, 3:4, 0:1], op=MUL)
        nc.vector.tensor_scalar_mul(out=mv3[:, 2, 0:1], in0=ssum2[:, 0:1],
                                    scalar1=1.0 / N)
        nc.vector.tensor_tensor(out=mv3[:, 2, 2:3], in0=mv3[:, 2, 0:1],
                                in1=mv3[:, 2, 0:1], op=MUL)
        with nc.allow_low_precision(reason="variance from moments"):
            nc.vector.tensor_scalar(out=mv3[:, 2, 1:2], in0=ssum2[:, 1:2],
                                    scalar1=1.0 / N, scalar2=None, op0=MUL)
        nc.vector.tensor_tensor(out=mv3[:, 2, 1:2], in0=mv3[:, 2, 1:2],
                                in1=mv3[:, 2, 2:3], op=SUB)
        psg = bsh.tile([8, NCO, 3], F32, tag="b", name="psg")
        nc.tensor.matmul(psg.rearrange("g co s -> g (co s)"), gmat_sb,
                         mv3.rearrange("p co s -> p (co s)"),
                         start=True, stop=True)
        nc.vector.tensor_copy(out=gsm[:], in_=psg[:])
        nc.vector.tensor_scalar_mul(out=gmu84[:], in0=gsm[:, :, 0], scalar1=1.0 / 16.0)
        nc.vector.tensor_tensor(out=gvar84[:], in0=gsm[:, :, 1], in1=gsm[:, :, 2],
                                op=ADD)
        nc.vector.tensor_scalar_mul(out=gvar84[:], in0=gvar84[:], scalar1=1.0 / 16.0)
        nc.vector.tensor_tensor(out=srt84[:], in0=gmu84[:], in1=gmu84[:], op=MUL)
        nc.vector.tensor_tensor(out=gvar84[:], in0=gvar84[:], in1=srt84[:], op=SUB)
        # rstd = exp(-0.5*ln(var+eps)): stays on the Ln+Exp activation set
        nc.scalar.activation(out=srt84[:], in_=gvar84[:], func=AF.Ln, bias=eps8[:])
        nc.scalar.activation(out=grs[:, :, 0], in_=srt84[:], func=AF.Exp,
                             scale=-0.5)
        nc.vector.tensor_copy(out=grs[:, :, 1], in_=gmu84[:])
        psr = bsh.tile([128, NCO, 2], F32, tag="b", name="psr")
        nc.tensor.matmul(psr.rearrange("p co s -> p (co s)"), gmatT_sb[:],
                         grs.rearrange("g co s -> g (co s)"),
                         start=True, stop=True)
        nc.vector.tensor_copy(out=rg_pc[:], in_=psr[:])
        nc.vector.tensor_tensor(out=a_pc[:], in0=gnw_pc, in1=rg_pc[:, :, 0],
                                op=MUL)
        nc.vector.tensor_tensor(out=tmp_pc[:], in0=rg_pc[:, :, 1], in1=a_pc[:], op=MUL)
        nc.vector.tensor_tensor(out=d_pc[:], in0=gnb_pc, in1=tmp_pc[:], op=SUB)

        # qbias = wq_h @ d + bq ; wqs = wqT * a (columns scaled per channel)
        qb = bsh.tile([DH, 1], F32, tag="b", name="qb")
        for co in range(NCO):
            nc.tensor.matmul(qb[:], wqT_sb[:, co, :], d_pc[:, co:co + 1],
                             start=(co == 0), stop=(co == NCO - 1))
        nc.vector.tensor_tensor(out=qbias[:], in0=qb[:], in1=bq_sb, op=ADD)
        for co in range(NCO):
            nc.vector.tensor_scalar_mul(out=wqs_bf[:, co, :], in0=wqT_sb[:, co, :],
                                        scalar1=a_pc[:, co:co + 1])

        # ---------------- Q projection ----------------
        def q_proj(ic):
            sl = slice(ic * IC, (ic + 1) * IC)
            pq = bsh.tile([DH, IC], F32, tag="b", name="pq")
            for co in range(NCO):
                nc.tensor.matmul(pq[:], wqs_bf[:, co, :], xb_sb[:, co, sl],
                                 start=(co == 0), stop=(co == NCO - 1))
            with nc.allow_low_precision(reason="fp8 attention operand"):
                nc.vector.tensor_scalar(out=q8[:, sl], in0=pq[:],
                                        scalar1=qbias[:], scalar2=None, op0=ADD)

        # ---------------- attention blocks ----------------
        pav_tiles = {}
        av_pending = []
        deferred = []  # finish-epilogue pieces, pumped between groups

        def pump():
            if deferred:
                deferred.pop(0)()

        def emit_av(pav, g, ee):
            nc.tensor.matmul(pav[:], vT8[:, g, :, :], ee[:],
                             start=(g == 0), stop=(g == NG - 1),
                             perf_mode=DR)

        def att_block(ic, qt):
            """QK + exp for chunk ic over quarter qt's key tiles; the AV
            matmuls trail by AV_LAG groups (drained at qt==3)."""
            sl = slice(ic * IC, (ic + 1) * IC)
            if qt == 0:
                pav_tiles[ic] = psav.tile([96, IC], F32, tag="pav", name="pav")
            pav = pav_tiles[ic]
            if pool_prelude:
                pool_prelude.pop(0)()
            for g in range(qt * (NG // 4), (qt + 1) * (NG // 4)):
                pump()
                jA, jB = 2 * g, 2 * g + 1
                tt = pst.tile([128, 2, IC], F32, tag="tps", name="tps")
                nc.tensor.matmul(tt[:, 0, :],
                                 bc2(k8[:, jA * 128:(jA + 1) * 128], DH, 128),
                                 bc2(q8[:, sl], DH, IC),
                                 start=True, stop=True, perf_mode=DR)
                nc.tensor.matmul(tt[:, 1, :],
                                 bc2(k8[:, jB * 128:(jB + 1) * 128], DH, 128),
                                 bc2(q8[:, sl], DH, IC),
                                 start=True, stop=True, perf_mode=DR)
                ee = expp.tile([128, 2, IC], FP8, tag="exp", name="exp")
                if g in TRICK_GROUPS:
                    bf = bfp.tile([128, 2, IC], F32, tag="bf", name="bf")
                    nc.vector.tensor_scalar(out=bf[:], in0=tt[:],
                                            scalar1=TRICK_A, scalar2=TRICK_B,
                                            op0=MUL, op1=ADD)
                    with nc.allow_low_precision(reason="bit-trick exp"):
                        nc.gpsimd.tensor_copy(out=ee[:].bitcast(U8), in_=bf[:])
                else:
                    nc.scalar.activation(out=ee[:], in_=tt[:], func=AF.Exp,
                                         scale=EXP_SCALE)
                av_pending.append((pav, g, ee))
                if len(av_pending) > AV_LAG:
                    emit_av(*av_pending.pop(0))
            if qt == 3:
                while av_pending:
                    emit_av(*av_pending.pop(0))

        def att_finish_head(ic):
            """normalize + attn8; the per-channel-block output projection
            pieces are queued on `deferred` and pumped between later groups
            so the DVE work spreads out instead of bursting."""
            sl = slice(ic * IC, (ic + 1) * IC)
            pav = pav_tiles.pop(ic)
            rden = attp.tile([1, IC], F32R, tag="rden", name="rden")
            with nc.allow_low_precision(reason="f32r matmul operand"):
                nc.vector.reciprocal(out=rden[:], in_=pav[64:65, :])
            rbp = bsh.tile([DH, IC], F32, tag="b", name="rbp")
            nc.tensor.matmul(rbp[:], ones_c0[:], rden[:], start=True, stop=True)
            rb = attp.tile([DH, IC], F32, tag="rb", name="rb")
            nc.vector.tensor_copy(out=rb[:], in_=rbp[:])
            attn8 = attp.tile([DH, IC], FP8, tag="attn", name="attn")
            with nc.allow_low_precision(reason="fp8 attention operand"):
                nc.vector.tensor_tensor(out=attn8[:], in0=pav[0:DH, :],
                                        in1=rb[:], op=MUL)

            def piece(ot, attn8=attn8, sl=sl):
                po = bsh.tile([128, IC], F32, tag="b", name="po")
                nc.tensor.matmul(po[:],
                                 bc2(woT8_sb[:, ot * 128:(ot + 1) * 128], DH, 128),
                                 bc2(attn8[:], DH, IC),
                                 start=True, stop=True, perf_mode=DR)
                pt = outp.tile([128, IC], BF16, tag="pt", name="pt")
                with nc.allow_low_precision(reason="bf16 partial output"):
                    nc.vector.scalar_tensor_tensor(out=pt[:], in0=po[:],
                                                   scalar=O_SCALE,
                                                   in1=xm_sb[:, ot, sl],
                                                   op0=MUL, op1=ADD)
                nc.sync.dma_start(out=pvw[:, ot, sl], in_=pt[:])

            for ot in range(NCO):
                deferred.append(lambda ot=ot: piece(ot))

        # ---------------- emission schedule ----------------
        q_proj(0)
        kv_evac(kv_tiles[1])
        q_proj(1)
        kv_evac(kv_tiles[2])
        kv_evac(kv_tiles[3])
        for qt in range(4):
            att_block(0, qt)
        q_proj(2)
        for qt in range(4):
            att_block(1, qt)
        q_proj(3)
        nq = 4
        for ic in range(2, NIC):
            att_finish_head(ic - 2)
            if nq < NIC:
                q_proj(nq)
                nq += 1
            if ic < NIC - 1:
                for qt in range(4):
                    att_block(ic, qt)
            else:
                att_block(ic, 0)
                att_block(ic, 1)
                att_finish_head(ic - 1)  # chunk 6 finishes amid chunk 7
                att_block(ic, 2)
                att_block(ic, 3)
        while deferred:
            pump()
        att_finish_head(NIC - 1)
        while deferred:
            pump()

        for p in [stats_pool, outp, attp, bfp, expp, bsh, psav, pst, singles]:
            p.release()

    nc.compile()
    return nc


GMAT = (np.arange(128)[:, None] // 16 == np.arange(8)[None, :]).astype(np.float32)
GMATT = np.ascontiguousarray(GMAT.T)
NP8 = mybir.dt.np(FP8)
NPBF = mybir.dt.np(BF16)
VCOL = np.zeros((128, 32), dtype=np.float32)
VCOL[:, 0] = 1.0
VCOL = VCOL.astype(NP8)

_NC_CACHE = None


def get_nc():
    global _NC_CACHE
    if _NC_CACHE is None:
        _NC_CACHE = build_nc()
    return _NC_CACHE


def kernel(x, context, gn_w, gn_b, wq, bq, wk, bk, wv, bv, wo, bo):
    from concourse.bass_utils import run_bass_kernel_spmd

    x = np.asarray(x, dtype=np.float32)
    context = np.asarray(context, dtype=np.float32)
    gn_w = np.asarray(gn_w, dtype=np.float32)
    gn_b = np.asarray(gn_b, dtype=np.float32)
    wq = np.asarray(wq, dtype=np.float32)
    bq = np.asarray(bq, dtype=np.float32)
    wk = np.asarray(wk, dtype=np.float32)
    bk = np.asarray(bk, dtype=np.float32)
    wv = np.asarray(wv, dtype=np.float32)
    bv = np.asarray(bv, dtype=np.float32)
    wo = np.asarray(wo, dtype=np.float32)
    bo = np.asarray(bo, dtype=np.float32)

    B, C, H, W = x.shape
    xb2 = np.ascontiguousarray(x.reshape(C, H * W)).astype(NPBF)
    ctx82 = np.ascontiguousarray(context.reshape(CTXC, H * W)).astype(NP8)

    in_maps = []
    for h in range(NH):
        hs = slice(h * DH, (h + 1) * DH)
        pa = np.zeros((128, 21), dtype=np.float32)
        pa[:, 0:4] = gn_w.reshape(NCO, 128).T
        pa[:, 4:8] = gn_b.reshape(NCO, 128).T
        bomv_h = bo * (1.0 if h == 0 else 0.0) + wo[:, hs] @ bv[hs]
        pa[:, 8:12] = bomv_h.reshape(NCO, 128).T
        pa[:, 12:20] = GMAT
        pa[:, 20] = 1.0 if h == 0 else 0.0
        pb = np.stack([bq[hs], bk[hs] * W8], axis=1).astype(np.float32)
        in_maps.append({
            "xb": xb2,
            "ctx8": ctx82,
            "wqT": np.ascontiguousarray(wq[hs, :].T),
            "wkT8": np.ascontiguousarray(wk[hs, :].T * W8).astype(NP8),
            "wvT8": np.ascontiguousarray(wv[hs, :].T * W8).astype(NP8),
            "woT8": np.ascontiguousarray(wo[:, hs].T * W8).astype(NP8),
            "packA": pa,
            "packB": np.ascontiguousarray(pb),
            "gmatT": GMATT,
            "vcol": VCOL,
        })

    nc = get_nc()
    res = run_bass_kernel_spmd(nc, in_maps, core_ids=list(range(NH)))
    acc = np.zeros((C, H * W), dtype=np.float64)
    for h in range(NH):
        acc += res.results[h]["partial"].astype(np.float64)
    return acc.astype(np.float32).reshape(B, C, H, W)
